# revision 14
# baseline (speedup 1.0000x reference)
"""Trainium2 Bass kernel for nn_ActionQuantizer (vq_codebook).

Self-contained: hardcodes shapes/sharding. Accepts FULL inputs, returns FULL outputs.
Data-parallel over batch N across 8 cores; weights/codebook replicated.
"""
import sys
sys.path.insert(0, "/opt/trn_rl_repo")

import numpy as np
import concourse.bass as bass
from concourse import bacc
import concourse.mybir as mybir
from concourse.tile import TileContext
from concourse.bass_utils import run_bass_kernel_spmd

# ---- problem constants ----
N = 131072
ACT_D = 12
COND_D = 256
K = 256            # codebook size
E = 16             # embedding dim
H1, H2 = 512, 256
COMMIT = 0.25
TAU = 0.07

NCORES = 8
NLOC = N // NCORES          # 16384
CH = 512                    # batch chunk (moving dim)
NCHUNK = NLOC // CH         # 32
ASUB = 4096                 # phase-A subsample columns (chunks 0..7)
NEWTON_ITERS = 4

f32 = mybir.dt.float32
f32r = mybir.dt.float32r
bf16 = mybir.dt.bfloat16
f16 = mybir.dt.float16
i32 = mybir.dt.int32
i16 = mybir.dt.int16
u32 = mybir.dt.uint32
Alu = mybir.AluOpType
Act = mybir.ActivationFunctionType

DEBUG = False
TRACE = False

# ---------------------------------------------------------------- host helpers

def _split_f32r(x):
    """Lossless split x = hi + lo with both f32r-legal (low 12 mantissa bits zero)."""
    x = np.ascontiguousarray(x, dtype=np.float32)
    b = x.view(np.uint32)
    hi = (b & np.uint32(0xFFFFF000)).view(np.float32)
    lo = (x - hi).astype(np.float32)
    return hi, lo


def _bf16(x):
    import ml_dtypes
    return np.ascontiguousarray(x, dtype=ml_dtypes.bfloat16).astype(np.float32)


def _f16_raw(x):
    return np.ascontiguousarray(np.asarray(x, dtype=np.float32).astype(np.float16))


# ---------------------------------------------------------------- kernel build

_CACHED_NC = None

def _build():
    global _CACHED_NC
    if _CACHED_NC is not None:
        return _CACHED_NC
    nc = bacc.Bacc("TRN2", target_bir_lowering=False, num_devices=NCORES)

    # ---- dram inputs (per-core shard + replicated weights) ----
    # x^T = [cond(256); act(12); ones(1)] hi/lo, f32r-rounded values
    xh_d = nc.dram_tensor("xh", [COND_D + ACT_D + 1, NLOC], f32r, kind="ExternalInput")
    xl_d = nc.dram_tensor("xl", [COND_D + ACT_D + 1, NLOC], f32r, kind="ExternalInput")
    w1h_d = nc.dram_tensor("w1h", [COND_D + ACT_D + 1, H1], f32r, kind="ExternalInput")
    w1l_d = nc.dram_tensor("w1l", [COND_D + ACT_D + 1, H1], f32r, kind="ExternalInput")
    w2h_d = nc.dram_tensor("w2h", [H1, H2], f32r, kind="ExternalInput")
    w2l_d = nc.dram_tensor("w2l", [H1, H2], f32r, kind="ExternalInput")
    w3h_d = nc.dram_tensor("w3h", [H2, E], f32r, kind="ExternalInput")
    w3l_d = nc.dram_tensor("w3l", [H2, E], f32r, kind="ExternalInput")
    b2p_d = nc.dram_tensor("b2p", [H2, 1], f32, kind="ExternalInput")
    b3p_d = nc.dram_tensor("b3p", [E, 1], f32, kind="ExternalInput")
    wTh_d = nc.dram_tensor("wTh", [E, K], f32r, kind="ExternalInput")   # normalized codebook^T hi
    wTl_d = nc.dram_tensor("wTl", [E, K], f32r, kind="ExternalInput")
    embT_d = nc.dram_tensor("embT", [E, K], f32, kind="ExternalInput")  # raw codebook^T
    ones16_d = nc.dram_tensor("ones16", [E, 1], f32r, kind="ExternalInput")
    dw1_d = nc.dram_tensor("dw1", [COND_D + E, H2], f16, kind="ExternalInput")  # [cond; q]
    db1_d = nc.dram_tensor("db1", [H2, 1], f32, kind="ExternalInput")
    dw2_d = nc.dram_tensor("dw2", [H2, H1], f16, kind="ExternalInput")
    dw3_d = nc.dram_tensor("dw3", [H1, ACT_D], f16, kind="ExternalInput")
    db2p_d = nc.dram_tensor("db2p", [H1, 1], f32, kind="ExternalInput")
    db3p_d = nc.dram_tensor("db3p", [ACT_D, 1], f32, kind="ExternalInput")

    # ---- dram outputs ----
    recon_d = nc.dram_tensor("reconT", [ACT_D, NLOC], f32, kind="ExternalOutput")
    qT_d = nc.dram_tensor("qT", [E, NLOC], f32, kind="ExternalOutput")
    idx_d = nc.dram_tensor("idx", [NLOC], i32, kind="ExternalOutput")
    cstats_d = nc.dram_tensor("cstats", [4, K], f32, kind="ExternalOutput")
    lsums_d = nc.dram_tensor("lsums", [32], f32, kind="ExternalOutput")
    if DEBUG:
        zdbg_d = nc.dram_tensor("zdbg", [E, NLOC], f32, kind="ExternalOutput")
        ddbg_d = nc.dram_tensor("ddbg", [K, NLOC], f32, kind="ExternalOutput")

    # ---- collective buffers (internal dram) ----
    cc_in = nc.dram_tensor("cc_in", [K, 2], f32)
    cc_out = nc.dram_tensor("cc_out", [K, 2], f32, addr_space="Shared")

    with TileContext(nc) as tc:
        import contextlib
        ctx = contextlib.ExitStack()
        with ctx:
            singles = ctx.enter_context(tc.tile_pool(name="singles", bufs=1))
            resident = ctx.enter_context(tc.tile_pool(name="resident", bufs=1))
            chunkio = ctx.enter_context(tc.tile_pool(name="chunkio", bufs=2))
            work = ctx.enter_context(tc.tile_pool(name="work", bufs=1))
            small = ctx.enter_context(tc.tile_pool(name="small", bufs=1))
            pm = ctx.enter_context(tc.tile_pool(name="pm", bufs=3, space="PSUM"))
            psml = ctx.enter_context(tc.tile_pool(name="psml", bufs=2, space="PSUM"))

            # ---- load weights once ----
            def load(pool, dram, shape, dtype, tag):
                t = pool.tile(shape, dtype, tag=tag, name=tag)
                nc.sync.dma_start(out=t, in_=dram.ap())
                return t

            # encoder weights as K-tiles [128, ...]
            w1h = singles.tile([128, 3, H1], f32r, tag="w1h", name="w1h")
            w1l = singles.tile([128, 3, H1], f32r, tag="w1l", name="w1l")
            for t in range(2):
                nc.sync.dma_start(out=w1h[:, t, :], in_=w1h_d[128 * t:128 * (t + 1), :])
                nc.sync.dma_start(out=w1l[:, t, :], in_=w1l_d[128 * t:128 * (t + 1), :])
            nc.sync.dma_start(out=w1h[0:13, 2, :], in_=w1h_d[256:269, :])
            nc.sync.dma_start(out=w1l[0:13, 2, :], in_=w1l_d[256:269, :])
            w2h = singles.tile([128, 4, H2], f32r, tag="w2h", name="w2h")
            w2l = singles.tile([128, 4, H2], f32r, tag="w2l", name="w2l")
            for t in range(4):
                nc.sync.dma_start(out=w2h[:, t, :], in_=w2h_d[128 * t:128 * (t + 1), :])
                nc.sync.dma_start(out=w2l[:, t, :], in_=w2l_d[128 * t:128 * (t + 1), :])
            w3h = singles.tile([128, 2, E], f32r, tag="w3h", name="w3h")
            w3l = singles.tile([128, 2, E], f32r, tag="w3l", name="w3l")
            for t in range(2):
                nc.sync.dma_start(out=w3h[:, t, :], in_=w3h_d[128 * t:128 * (t + 1), :])
                nc.sync.dma_start(out=w3l[:, t, :], in_=w3l_d[128 * t:128 * (t + 1), :])
            def load_scal(dram, T, tag):
                t = singles.tile([128, T, 1], f32, tag=tag, name=tag)
                nc.sync.dma_start(out=t, in_=bass.AP(tensor=dram, offset=0,
                                                     ap=[[1, 128], [128, T], [1, 1]]))
                return t

            b2p = load_scal(b2p_d, 2, "b2p")  # per-Mtile scalars
            b2p1 = singles.tile([128, 2, 1], f32, tag="b2p1", name="b2p1")
            nc.vector.tensor_scalar(out=b2p1, in0=b2p, scalar1=1.0, scalar2=None, op0=Alu.add)
            b3p = load(singles, b3p_d, [E, 1], f32, "b3p")
            wTh = load(singles, wTh_d, [E, K], f32r, "wTh")
            wTl = load(singles, wTl_d, [E, K], f32r, "wTl")
            embT = load(singles, embT_d, [E, K], f32, "embT")
            ones16 = load(singles, ones16_d, [E, 1], f32r, "ones16")
            dw1 = singles.tile([128, 3, H2], f16, tag="dw1", name="dw1")
            for t in range(2):
                nc.sync.dma_start(out=dw1[:, t, :], in_=dw1_d[128 * t:128 * (t + 1), :])
            nc.sync.dma_start(out=dw1[0:16, 2, :], in_=dw1_d[256:272, :])
            dw2 = singles.tile([128, 2, H1], f16, tag="dw2", name="dw2")
            for t in range(2):
                nc.sync.dma_start(out=dw2[:, t, :], in_=dw2_d[128 * t:128 * (t + 1), :])
            dw3 = singles.tile([128, 4, ACT_D], f16, tag="dw3", name="dw3")
            for t in range(4):
                nc.sync.dma_start(out=dw3[:, t, :], in_=dw3_d[128 * t:128 * (t + 1), :])
            db1 = load_scal(db1_d, 2, "db1")
            db1p1 = singles.tile([128, 2, 1], f32, tag="db1p1", name="db1p1")
            nc.vector.tensor_scalar(out=db1p1, in0=db1, scalar1=1.0, scalar2=None, op0=Alu.add)
            db2p = load_scal(db2p_d, 4, "db2p")
            db2p1 = singles.tile([128, 4, 1], f32, tag="db2p1", name="db2p1")
            nc.vector.tensor_scalar(out=db2p1, in0=db2p, scalar1=1.0, scalar2=None, op0=Alu.add)
            db3p = load(singles, db3p_d, [ACT_D, 1], f32, "db3p")

            # resident code-major distances (bf16): A = first ASUB cols, B = rest
            dcmA = resident.tile([128, 2, ASUB], bf16, tag="dcmA", name="dcmA")
            dcmB = resident.tile([128, 2, NLOC - ASUB], bf16, tag="dcmB", name="dcmB")
            # per-chunk loss accumulators
            qsums = singles.tile([E, NCHUNK], f32, tag="qsums", name="qsums")
            rsums = singles.tile([ACT_D, NCHUNK], f32, tag="rsums", name="rsums")

            # ================= main chunk loop =================
            for c in range(NCHUNK):
                cs0 = c * CH
                xh = []
                xl = []
                for t, rr in [(0, (0, 128)), (1, (128, 256)), (2, (256, 269))]:
                    nrow = rr[1] - rr[0]
                    th_ = chunkio.tile([nrow, CH], f32r, tag=f"xh{t}", name=f"xh{t}", bufs=1 if t == 2 else None)
                    nc.sync.dma_start(out=th_, in_=xh_d[rr[0]:rr[1], cs0:cs0 + CH])
                    tl_ = chunkio.tile([nrow, CH], f32r, tag=f"xl{t}", name=f"xl{t}", bufs=1 if t == 2 else None)
                    nc.sync.dma_start(out=tl_, in_=xl_d[rr[0]:rr[1], cs0:cs0 + CH])
                    xh.append(th_)
                    xl.append(tl_)

                # ---------- enc1: h1 = x @ W1  (4 Mtiles) ----------
                ps1 = [pm.tile([128, 2, CH], f32, tag="pm", name="pm"), pm.tile([128, 2, CH], f32, tag="pm", name="pm")]
                for m in range(4):
                    pv = ps1[m // 2][:, m % 2, :]
                    ms = slice(128 * m, 128 * (m + 1))
                    first = True
                    terms = []
                    for t in range(3):
                        kn = 13 if t == 2 else 128
                        terms.append((w1h[0:kn, t, ms], xh[t]))
                        terms.append((w1l[0:kn, t, ms], xh[t]))
                        terms.append((w1h[0:kn, t, ms], xl[t]))
                    for i, (lw, rx) in enumerate(terms):
                        nc.tensor.matmul(pv, lw, rx, start=(i == 0), stop=(i == len(terms) - 1))

                # ---------- ELU1 (p = elu+1 = min(exp(s), max(s+1,1))) ----------
                e1 = work.tile([128, 4, CH], f32, tag="e1", name="e1")
                r1 = work.tile([128, 4, CH], f32, tag="r1", name="r1")
                hh1 = work.tile([128, 4, CH], f32r, tag="hh1", name="hh1")
                hl1 = work.tile([128, 4, CH], f32r, tag="e1", name="hl1")
                for half in range(2):
                    hs = slice(2 * half, 2 * half + 2)
                    nc.scalar.activation(e1[:, hs, :], ps1[half][:, :, :], Act.Exp)
                    nc.vector.tensor_scalar(out=r1[:, hs, :], in0=ps1[half][:, :, :],
                                            scalar1=1.0, scalar2=1.0, op0=Alu.add, op1=Alu.max)
                nc.vector.tensor_tensor(out=r1, in0=e1, in1=r1, op=Alu.min)
                nc.vector.tensor_copy(hh1, r1)
                nc.vector.tensor_tensor(out=hl1, in0=r1, in1=hh1.bitcast(f32), op=Alu.subtract)

                # ---------- enc2 ----------
                ps2 = pm.tile([128, 2, CH], f32, tag="pm", name="pm")
                for m in range(2):
                    pv = ps2[:, m, :]
                    ms = slice(128 * m, 128 * (m + 1))
                    terms = []
                    for t in range(4):
                        terms.append((w2h[:, t, ms], hh1[:, t, :]))
                        terms.append((w2l[:, t, ms], hh1[:, t, :]))
                        terms.append((w2h[:, t, ms], hl1[:, t, :]))
                    for i, (lw, rx) in enumerate(terms):
                        nc.tensor.matmul(pv, lw, rx, start=(i == 0), stop=(i == len(terms) - 1))

                # ---------- ELU2 (bias fold b2p) ----------
                e2 = work.tile([128, 2, CH], f32, tag="e2", name="e2")
                r2 = work.tile([128, 2, CH], f32, tag="r2", name="r2")
                hh2 = work.tile([128, 2, CH], f32r, tag="hh2", name="hh2")
                hl2 = work.tile([128, 2, CH], f32r, tag="e2", name="hl2")
                for m in range(2):
                    nc.scalar.activation(e2[:, m, :], ps2[:, m, :], Act.Exp, bias=b2p[:, m, :])
                    nc.vector.tensor_scalar(out=r2[:, m, :], in0=ps2[:, m, :],
                                            scalar1=b2p1[:, m, :], scalar2=1.0, op0=Alu.add, op1=Alu.max)
                nc.vector.tensor_tensor(out=r2, in0=e2, in1=r2, op=Alu.min)
                nc.vector.tensor_copy(hh2, r2)
                nc.vector.tensor_tensor(out=hl2, in0=r2, in1=hh2.bitcast(f32), op=Alu.subtract)

                # ---------- enc3: z ----------
                psz = psml.tile([E, CH], f32, tag="ps_s", name="psz")
                terms = []
                for t in range(2):
                    terms.append((w3h[:, t, :], hh2[:, t, :]))
                    terms.append((w3l[:, t, :], hh2[:, t, :]))
                    terms.append((w3h[:, t, :], hl2[:, t, :]))
                for i, (lw, rx) in enumerate(terms):
                    nc.tensor.matmul(psz, lw, rx, start=(i == 0), stop=(i == len(terms) - 1))
                z = small.tile([E, CH], f32, tag="z", name="z")
                nc.vector.tensor_scalar(out=z, in0=psz, scalar1=b3p[:, 0:1], scalar2=None, op0=Alu.add)
                if DEBUG:
                    nc.sync.dma_start(out=zdbg_d[:, cs0:cs0 + CH], in_=z)

                # ---------- normalize ----------
                zzr = small.tile([E, CH], f32r, tag="zzr", name="zzr")
                nc.vector.tensor_tensor(out=zzr, in0=z, in1=z, op=Alu.mult)
                psn = psml.tile([1, CH], f32, tag="ps_s", name="psn")
                nc.tensor.matmul(psn, ones16, zzr, start=True, stop=True)
                nrm = small.tile([1, CH], f32, tag="nrm", name="nrm")
                nc.scalar.activation(nrm, psn, Act.Sqrt)
                invn1 = small.tile([1, CH], f32, tag="invn1", name="invn1")
                nc.vector.reciprocal_approx_fast(invn1, nrm)
                invn16 = small.tile([E, CH], f32, tag="nrm", name="invn16")
                nc.gpsimd.partition_broadcast(invn16, invn1, channels=E)
                zhat = small.tile([E, CH], f32, tag="zhat", name="zhat")
                nc.vector.tensor_tensor(out=zhat, in0=z, in1=invn16, op=Alu.mult)
                zh = small.tile([E, CH], f32r, tag="zh", name="zh")
                nc.vector.tensor_copy(zh, zhat)
                zl = small.tile([E, CH], f32r, tag="zl", name="zl")
                nc.vector.tensor_tensor(out=zl, in0=zhat, in1=zh.bitcast(f32), op=Alu.subtract)

                # ---------- distances code-major (selection, 1-term) ----------
                pscm = pm.tile([128, 2, CH], f32, tag="pm", name="pm")
                for mcode in range(2):
                    nc.tensor.matmul(pscm[:, mcode, :], wTh[:, 128 * mcode:128 * (mcode + 1)], zh,
                                     start=True, stop=True)
                if c * CH < ASUB:
                    nc.scalar.copy(dcmA[:, :, cs0:cs0 + CH], pscm)
                else:
                    nc.scalar.copy(dcmB[:, :, cs0 - ASUB:cs0 - ASUB + CH], pscm)
                if DEBUG:
                    dcmf = small.tile([128, 2, CH], f32, tag="dcmf", name="dcmf")
                    nc.vector.tensor_copy(dcmf, pscm)
                    nc.sync.dma_start(out=bass.AP(tensor=ddbg_d, offset=cs0,
                                                  ap=[[NLOC, 128], [128 * NLOC, 2], [1, CH]]),
                                      in_=dcmf)

                # ---------- distances batch-major (argmax, 3-term) ----------
                psbm = pm.tile([128, 4, 256], f32, tag="pm", name="pm")
                for j in range(4):
                    js = slice(128 * j, 128 * (j + 1))
                    terms = [(zh[:, js], wTh), (zl[:, js], wTh), (zh[:, js], wTl)]
                    for i, (lw, rx) in enumerate(terms):
                        nc.tensor.matmul(psbm[:, j, :], lw, rx, start=(i == 0), stop=(i == 2))
                m8 = small.tile([128, 4, 8], f32, tag="m8", name="m8")
                i8 = small.tile([128, 4, 8], u32, tag="i8", name="i8")
                for j in range(4):
                    nc.vector.max(m8[:, j, :], psbm[:, j, :])
                    nc.vector.max_index(i8[:, j, :], m8[:, j, :], psbm[:, j, :])
                idx32 = small.tile([128, 4], i32, tag="idx32", name="idx32")
                nc.vector.tensor_copy(idx32, i8[:, :, 0])
                nc.sync.dma_start(
                    out=bass.AP(tensor=idx_d, offset=cs0, ap=[[1, 128], [128, 4]]),
                    in_=idx32)
                idxw32 = small.tile([16, CH // 16], i32, tag="idxw32", name="idxw32")
                nc.sync.dma_start(
                    out=idxw32,
                    in_=bass.AP(tensor=idx_d, offset=cs0, ap=[[1, 16], [16, CH // 16]]))
                idxw = small.tile([16, CH // 16], i16, tag="idxw", name="idxw")
                nc.vector.tensor_copy(idxw, idxw32)

                # ---------- gather quantized ----------
                gsb = small.tile([E, CH], f32, tag="zhat", name="gsb")
                nc.gpsimd.ap_gather(gsb, embT, idxw, channels=16, num_elems=K, d=1, num_idxs=CH)
                nc.sync.dma_start(out=qT_d[:, cs0:cs0 + CH], in_=gsb)
                qd = small.tile([E, CH], f32, tag="zzr", name="qd")
                nc.vector.tensor_tensor(out=qd, in0=gsb, in1=z, op=Alu.subtract)
                nc.scalar.activation(qd, qd, Act.Square, accum_out=qsums[:, c:c + 1])

                # ---------- decoder rhs ----------
                cb0 = work.tile([128, CH], f16, tag="cb0", name="cb0")
                nc.vector.tensor_copy(cb0, xh[0].bitcast(f32))
                cb1 = work.tile([128, CH], f16, tag="cb1", name="cb1")
                nc.vector.tensor_copy(cb1, xh[1].bitcast(f32))
                dr2 = work.tile([16, CH], f16, tag="dr2", name="dr2")
                nc.vector.tensor_copy(dr2, gsb)

                # ---------- dec1 ----------
                psd1 = pm.tile([128, 2, CH], f32, tag="pm", name="pm")
                for m in range(2):
                    ms = slice(128 * m, 128 * (m + 1))
                    terms = [(dw1[:, 0, ms], cb0), (dw1[:, 1, ms], cb1), (dw1[0:16, 2, ms], dr2)]
                    for i, (lw, rx) in enumerate(terms):
                        nc.tensor.matmul(psd1[:, m, :], lw, rx, start=(i == 0), stop=(i == 2))
                ed1 = work.tile([128, 2, CH], f16, tag="ed1", name="ed1")
                rd1 = work.tile([128, 2, CH], f16, tag="rd1", name="rd1")
                for m in range(2):
                    nc.scalar.activation(ed1[:, m, :], psd1[:, m, :], Act.Exp, bias=db1[:, m, :])
                    nc.vector.tensor_scalar(out=rd1[:, m, :], in0=psd1[:, m, :],
                                            scalar1=db1p1[:, m, :], scalar2=1.0, op0=Alu.add, op1=Alu.max)
                nc.vector.tensor_tensor(out=rd1, in0=ed1, in1=rd1, op=Alu.min)

                # ---------- dec2 ----------
                psd2 = [pm.tile([128, 2, CH], f32, tag="pm", name="pm"), pm.tile([128, 2, CH], f32, tag="pm", name="pm")]
                for m in range(4):
                    pv = psd2[m // 2][:, m % 2, :]
                    ms = slice(128 * m, 128 * (m + 1))
                    for t in range(2):
                        nc.tensor.matmul(pv, dw2[:, t, ms], rd1[:, t, :], start=(t == 0), stop=(t == 1))
                ed2 = work.tile([128, 4, CH], f16, tag="ed2", name="ed2")
                rd2 = work.tile([128, 4, CH], f16, tag="rd2", name="rd2")
                for m in range(4):
                    nc.scalar.activation(ed2[:, m, :], psd2[m // 2][:, m % 2, :], Act.Exp,
                                         bias=db2p[:, m, :])
                    nc.vector.tensor_scalar(out=rd2[:, m, :], in0=psd2[m // 2][:, m % 2, :],
                                            scalar1=db2p1[:, m, :], scalar2=1.0,
                                            op0=Alu.add, op1=Alu.max)
                nc.vector.tensor_tensor(out=rd2, in0=ed2, in1=rd2, op=Alu.min)

                # ---------- dec3 ----------
                psd3 = psml.tile([ACT_D, CH], f32, tag="ps_s", name="psd3")
                for t in range(4):
                    nc.tensor.matmul(psd3, dw3[:, t, 0:ACT_D], rd2[:, t, :], start=(t == 0), stop=(t == 3))
                recon = small.tile([ACT_D, CH], f32, tag="recon", name="recon")
                nc.vector.tensor_scalar(out=recon, in0=psd3, scalar1=db3p[:, 0:1], scalar2=None, op0=Alu.add)
                nc.sync.dma_start(out=recon_d[:, cs0:cs0 + CH], in_=recon)
                rdf = small.tile([ACT_D, CH], f32, tag="z", name="rdf")
                nc.vector.tensor_tensor(out=rdf, in0=recon, in1=xh[2][0:12, :].bitcast(f32), op=Alu.subtract)
                nc.vector.tensor_tensor(out=rdf, in0=rdf, in1=xl[2][0:12, :].bitcast(f32), op=Alu.subtract)
                nc.scalar.activation(rdf, rdf, Act.Square, accum_out=rsums[:, c:c + 1])

            # ================= selection =================
            sel = ctx.enter_context(tc.tile_pool(name="sel", bufs=1))
            selps = ctx.enter_context(tc.tile_pool(name="selps", bufs=2))

            # ---- phase A: local quantile newton on dcmA ----
            s1 = sel.tile([128, 2], f32, tag="s1", name="s1")
            s2 = sel.tile([128, 2], f32, tag="s2", name="s2")
            s2c = sel.tile([128, 2, 2], f32, tag="s2c", name="s2c")
            for kt in range(2):
                dummy = selps.tile([128, ASUB], bf16, tag="scr", name="dummy")
                nc.vector.tensor_scalar(out=dummy, in0=dcmA[:, kt, :], scalar1=0.0, scalar2=0.0,
                                        op0=Alu.add, op1=Alu.add, accum_out=s1[:, kt:kt + 1])
                for h in range(2):
                    sq = selps.tile([128, ASUB // 2], f32, tag="scr", name="sq")
                    nc.scalar.activation(sq, dcmA[:, kt, h * (ASUB // 2):(h + 1) * (ASUB // 2)],
                                         Act.Square, accum_out=s2c[:, kt, h:h + 1])
            nc.vector.tensor_reduce(out=s2, in_=s2c, axis=mybir.AxisListType.X, op=Alu.add)
            mu = sel.tile([128, 2], f32, tag="mu", name="mu")
            nc.vector.tensor_scalar(out=mu, in0=s1, scalar1=1.0 / ASUB, scalar2=None, op0=Alu.mult)
            var = sel.tile([128, 2], f32, tag="var", name="var")
            nc.vector.tensor_scalar(out=var, in0=s2, scalar1=1.0 / ASUB, scalar2=None, op0=Alu.mult)
            mumu = sel.tile([128, 2], f32, tag="mumu", name="mumu")
            nc.vector.tensor_tensor(out=mumu, in0=mu, in1=mu, op=Alu.mult)
            nc.vector.tensor_tensor(out=var, in0=var, in1=mumu, op=Alu.subtract)
            sd = sel.tile([128, 2], f32, tag="sd", name="sd")
            nc.scalar.activation(sd, var, Act.Sqrt)
            invsd = sel.tile([128, 2], f32, tag="invsd", name="invsd")
            nc.vector.reciprocal_approx_fast(invsd, sd)
            sdh = sel.tile([128, 2], f32, tag="sdh", name="sdh")   # step clamp = 0.5 sd
            nc.vector.tensor_scalar(out=sdh, in0=sd, scalar1=0.5, scalar2=None, op0=Alu.mult)
            sdhn = sel.tile([128, 2], f32, tag="sdhn", name="sdhn")
            nc.vector.tensor_scalar(out=sdhn, in0=sdh, scalar1=-1.0, scalar2=None, op0=Alu.mult)

            th = sel.tile([128, 2], f32, tag="th", name="th")
            tsc = sel.tile([128, 2], f32, tag="tsc", name="tsc")
            nc.vector.tensor_scalar(out=tsc, in0=sd, scalar1=2.653, scalar2=None, op0=Alu.mult)
            nc.vector.tensor_tensor(out=th, in0=mu, in1=tsc, op=Alu.add)
            tl = sel.tile([128, 2], f32, tag="tl", name="tl")
            nc.vector.tensor_copy(tl, mu)

            cnt = sel.tile([128, 2], f32, tag="cnt", name="cnt")
            targ_hi = ASUB * 512.0 / N       # 16
            targ_lo = ASUB * 0.5             # 2048
            for it in range(NEWTON_ITERS):
                for (tt_, cmp_op, targ) in [(th, Alu.is_gt, targ_hi), (tl, Alu.is_lt, targ_lo)]:
                    for kt in range(2):
                        dummy = selps.tile([128, ASUB], bf16, tag="scr", name="dummy")
                        nc.vector.tensor_scalar(out=dummy, in0=dcmA[:, kt, :],
                                                scalar1=tt_[:, kt:kt + 1], scalar2=0.0,
                                                op0=cmp_op, op1=Alu.add, accum_out=cnt[:, kt:kt + 1])
                    # newton update: t += sign * (cnt - targ) / (ASUB * pdf(t))
                    u = sel.tile([128, 2], f32, tag="u", name="u")
                    nc.vector.tensor_tensor(out=u, in0=tt_, in1=mu, op=Alu.subtract)
                    nc.vector.tensor_tensor(out=u, in0=u, in1=invsd, op=Alu.mult)
                    nc.vector.tensor_tensor(out=u, in0=u, in1=u, op=Alu.mult)
                    pdf = sel.tile([128, 2], f32, tag="pdf", name="pdf")
                    nc.scalar.activation(pdf, u, Act.Exp, scale=-0.5)
                    nc.vector.tensor_tensor(out=pdf, in0=pdf, in1=invsd, op=Alu.mult)
                    nc.vector.tensor_scalar(out=pdf, in0=pdf, scalar1=0.3989423 * ASUB, scalar2=None,
                                            op0=Alu.mult)
                    ipdf = sel.tile([128, 2], f32, tag="ipdf", name="ipdf")
                    nc.vector.reciprocal_approx_fast(ipdf, pdf)
                    step = sel.tile([128, 2], f32, tag="step", name="step")
                    nc.vector.tensor_scalar(out=step, in0=cnt, scalar1=float(targ), scalar2=None,
                                            op0=Alu.subtract)
                    nc.vector.tensor_tensor(out=step, in0=step, in1=ipdf, op=Alu.mult)
                    if cmp_op == Alu.is_lt:
                        nc.vector.tensor_scalar(out=step, in0=step, scalar1=-1.0, scalar2=None,
                                                op0=Alu.mult)
                    nc.vector.tensor_tensor(out=step, in0=step, in1=sdh, op=Alu.min)
                    nc.vector.tensor_tensor(out=step, in0=step, in1=sdhn, op=Alu.max)
                    nc.vector.tensor_tensor(out=tt_, in0=tt_, in1=step, op=Alu.add)

            # ---- allreduce thresholds ----
            tpack = sel.tile([128, 2, 2], f32, tag="tpack", name="tpack")
            nc.vector.tensor_copy(tpack[:, :, 0], th)
            nc.vector.tensor_copy(tpack[:, :, 1], tl)
            nc.sync.dma_start(
                out=bass.AP(tensor=cc_in, offset=0, ap=[[2, 128], [256, 2], [1, 2]]),
                in_=tpack)
            nc.gpsimd.collective_compute(
                "AllReduce", Alu.add,
                ins=[cc_in.ap()], outs=[cc_out.ap()],
                replica_groups=[list(range(NCORES))])
            tbar = sel.tile([128, 2, 2], f32, tag="tbar", name="tbar")
            nc.sync.dma_start(
                out=tbar,
                in_=bass.AP(tensor=cc_out, offset=0, ap=[[2, 128], [256, 2], [1, 2]]))
            nc.vector.tensor_scalar(out=tbar, in0=tbar, scalar1=1.0 / NCORES, scalar2=None, op0=Alu.mult)
            thb = sel.tile([128, 2], f32, tag="thb", name="thb")
            nc.vector.tensor_copy(thb, tbar[:, :, 0])
            tlb = sel.tile([128, 2], f32, tag="tlb", name="tlb")
            nc.vector.tensor_copy(tlb, tbar[:, :, 1])
            tlbs = sel.tile([128, 2], f32, tag="tlbs", name="tlbs")   # -t_lo / tau (exp bias)
            nc.vector.tensor_scalar(out=tlbs, in0=tlb, scalar1=-1.0 / TAU, scalar2=None, op0=Alu.mult)

            # ---- phase B: masked sums over full data ----
            BCH = 2048
            nbc = NLOC // BCH
            ahic = sel.tile([128, 2, nbc], f32, tag="ahic", name="ahic")
            aloc = sel.tile([128, 2, nbc], f32, tag="aloc", name="aloc")
            for g in range(nbc):
                g0 = g * BCH
                if g0 < ASUB:
                    dsl = dcmA[:, :, g0:g0 + BCH]
                else:
                    dsl = dcmB[:, :, g0 - ASUB:g0 - ASUB + BCH]
                exg = selps.tile([128, 2, BCH], bf16, tag="scr", name="exg")
                for kt in range(2):
                    nc.scalar.activation(exg[:, kt, :], dsl[:, kt, :], Act.Exp,
                                         bias=tlbs[:, kt:kt + 1], scale=1.0 / TAU)
                    dummy = selps.tile([128, BCH], bf16, tag="dummy2", name="dummy2")
                    nc.vector.tensor_scalar(out=dummy, in0=dsl[:, kt, :],
                                            scalar1=thb[:, kt:kt + 1], scalar2=0.0,
                                            op0=Alu.max, op1=Alu.add,
                                            accum_out=ahic[:, kt, g:g + 1])
                    dummy2 = selps.tile([128, BCH], bf16, tag="scr", name="dummy2")
                    nc.vector.tensor_scalar(out=dummy2, in0=exg[:, kt, :],
                                            scalar1=1.0, scalar2=0.0,
                                            op0=Alu.min, op1=Alu.add,
                                            accum_out=aloc[:, kt, g:g + 1])
            ahi = sel.tile([128, 2], f32, tag="ahi", name="ahi")
            nc.vector.tensor_reduce(out=ahi, in_=ahic, axis=mybir.AxisListType.X, op=Alu.add)
            alo = sel.tile([128, 2], f32, tag="alo", name="alo")
            nc.vector.tensor_reduce(out=alo, in_=aloc, axis=mybir.AxisListType.X, op=Alu.add)

            # ---- outputs: cstats + lsums ----
            for r, t in [(0, ahi), (1, alo), (2, thb), (3, tlb)]:
                nc.sync.dma_start(
                    out=bass.AP(tensor=cstats_d, offset=r * K, ap=[[1, 128], [128, 2]]),
                    in_=t)
            qtot = sel.tile([E, 1], f32, tag="qtot", name="qtot")
            nc.vector.tensor_reduce(out=qtot, in_=qsums, axis=mybir.AxisListType.X, op=Alu.add)
            rtot = sel.tile([ACT_D, 1], f32, tag="rtot", name="rtot")
            nc.vector.tensor_reduce(out=rtot, in_=rsums, axis=mybir.AxisListType.X, op=Alu.add)
            nc.sync.dma_start(out=bass.AP(tensor=lsums_d, offset=0, ap=[[1, E]]), in_=qtot)
            nc.sync.dma_start(out=bass.AP(tensor=lsums_d, offset=16, ap=[[1, ACT_D]]), in_=rtot)

    nc.compile()
    _CACHED_NC = nc
    return nc


# ---------------------------------------------------------------- host wrapper

def kernel(actions, conditions, enc_w1, enc_b1, enc_w2, enc_b2, enc_w3, enc_b3,
           dec_w1, dec_b1, dec_w2, dec_b2, dec_w3, dec_b3, embedding):
    actions = np.asarray(actions, dtype=np.float32)
    conditions = np.asarray(conditions, dtype=np.float32)
    enc_w1 = np.asarray(enc_w1, dtype=np.float32)
    enc_b1 = np.asarray(enc_b1, dtype=np.float32)
    enc_w2 = np.asarray(enc_w2, dtype=np.float32)
    enc_b2 = np.asarray(enc_b2, dtype=np.float32)
    enc_w3 = np.asarray(enc_w3, dtype=np.float32)
    enc_b3 = np.asarray(enc_b3, dtype=np.float32)
    dec_w1 = np.asarray(dec_w1, dtype=np.float32)
    dec_b1 = np.asarray(dec_b1, dtype=np.float32)
    dec_w2 = np.asarray(dec_w2, dtype=np.float32)
    dec_b2 = np.asarray(dec_b2, dtype=np.float32)
    dec_w3 = np.asarray(dec_w3, dtype=np.float32)
    dec_b3 = np.asarray(dec_b3, dtype=np.float32)
    embedding = np.asarray(embedding, dtype=np.float32)

    # ---- weight prep (shared across cores) ----
    # W1 rows reordered: [cond(256); act(12); bias(1)]
    W1 = np.concatenate([enc_w1[ACT_D:, :], enc_w1[:ACT_D, :], enc_b1[None, :]], 0)
    w1h, w1l = _split_f32r(W1)
    w2h, w2l = _split_f32r(enc_w2)
    w3h, w3l = _split_f32r(enc_w3)
    b2p = (enc_b2.astype(np.float64) - enc_w2.astype(np.float64).sum(0)).astype(np.float32)[:, None]
    b3p = (enc_b3.astype(np.float64) - enc_w3.astype(np.float64).sum(0)).astype(np.float32)[:, None]
    wn = embedding / np.maximum(np.linalg.norm(embedding, axis=1, keepdims=True), 1e-12)
    wTh, wTl = _split_f32r(np.ascontiguousarray(wn.T))
    embT = np.ascontiguousarray(embedding.T)
    ones16 = np.ones((E, 1), np.float32)
    # DW1 rows: [cond(256); q(16); bias(1)]
    DW1 = np.concatenate([dec_w1[E:, :], dec_w1[:E, :]], 0)
    dw1 = _f16_raw(DW1)
    dw2 = _f16_raw(dec_w2)
    dw3 = _f16_raw(dec_w3)
    db2p = (dec_b2.astype(np.float64) - dec_w2.astype(np.float16).astype(np.float64).sum(0)).astype(np.float32)[:, None]
    db3p = (dec_b3.astype(np.float64) - dec_w3.astype(np.float16).astype(np.float64).sum(0)).astype(np.float32)[:, None]

    shared = dict(w1h=w1h, w1l=w1l, w2h=w2h, w2l=w2l, w3h=w3h, w3l=w3l,
                  b2p=b2p, b3p=b3p, wTh=wTh, wTl=wTl, embT=embT, ones16=ones16,
                  dw1=dw1, dw2=dw2, dw3=dw3, db1=dec_b1[:, None].astype(np.float32), db2p=db2p, db3p=db3p)

    in_maps = []
    for i in range(NCORES):
        sl = slice(i * NLOC, (i + 1) * NLOC)
        xT = np.concatenate([conditions[sl].T, actions[sl].T, np.ones((1, NLOC), np.float32)], 0)
        xh, xl = _split_f32r(xT)
        m = dict(shared)
        m["xh"] = xh
        m["xl"] = xl
        in_maps.append(m)

    nc = _build()
    res = run_bass_kernel_spmd(nc, in_maps, core_ids=list(range(NCORES)), trace=TRACE)
    results = res.results
    kernel._last_exec_time_ns = res.exec_time_ns
    kernel._last_results = results

    # ---- host unshard / finish ----
    reconstructed = np.concatenate([r["reconT"].T for r in results], 0)
    quantized_st = np.concatenate([r["qT"].T for r in results], 0)
    idx = np.concatenate([r["idx"] for r in results], 0).astype(np.int32)

    qsum = np.sum([r["lsums"][0:16] for r in results], axis=(0, 1))
    rsum = np.sum([r["lsums"][16:28] for r in results], axis=(0, 1))
    q_latent = np.float32(qsum / (N * E))
    e_latent = np.float32(COMMIT * (qsum / (N * E)))
    recon_loss = np.float32(rsum / (N * ACT_D))

    counts = np.bincount(idx, minlength=K).astype(np.float64)
    avg = counts / N
    perplexity = np.float32(np.exp(-np.sum(avg * np.log(avg + 1e-10))))

    A_hi = np.sum([r["cstats"][0] for r in results], 0).astype(np.float64)
    A_lo = np.sum([r["cstats"][1] for r in results], 0).astype(np.float64)
    t_hi = results[0]["cstats"][2].astype(np.float64)
    t_lo = results[0]["cstats"][3].astype(np.float64)
    n_pos = N // K                                      # 512
    pos = (A_hi - (N - n_pos) * t_hi) / n_pos           # mean of top n_pos
    S = A_lo - N / 2.0                                  # sum exp((v - t_lo)/tau) over bottom half
    lse = t_lo / TAU + np.log(np.exp((pos - t_lo) / TAU) + S)
    contra = np.float32(np.mean(lse - pos / TAU))

    return (reconstructed, quantized_st, idx, q_latent, e_latent, contra,
            np.float32(perplexity), recon_loss)


kernel._last_exec_time_ns = None

if __name__ == "__main__":
    import reference
    inputs = reference.setup_inputs()
    outs = kernel(**{k: np.asarray(v) for k, v in inputs.items()})
    print("kernel ran; exec_time_ns:", kernel._last_exec_time_ns)


# revision 23
# speedup vs baseline: 1.2087x; 1.2087x over previous
"""Trainium2 Bass kernel for nn_ActionQuantizer (vq_codebook).

Self-contained: hardcodes shapes/sharding. Accepts FULL inputs, returns FULL outputs.
Data-parallel over batch N across 8 cores; weights/codebook replicated.

v2: K-stacked f32r 3-term matmuls, DRAM-resident selection matrix, deeper buffering.
"""
import sys
sys.path.insert(0, "/opt/trn_rl_repo")

import numpy as np
import concourse.bass as bass
from concourse import bacc
import concourse.mybir as mybir
from concourse.tile import TileContext
from concourse.bass_utils import run_bass_kernel_spmd

# ---- problem constants ----
N = 131072
ACT_D = 12
COND_D = 256
K = 256            # codebook size
E = 16             # embedding dim
H1, H2 = 512, 256
COMMIT = 0.25
TAU = 0.07

NCORES = 8
NLOC = N // NCORES          # 16384
CH = 512                    # batch chunk (moving dim)
NCHUNK = NLOC // CH         # 32
ASUB = 4096                 # phase-A subsample columns
NEWTON_ITERS = 4
XMIX_R = 44                 # packed [act_h;1;act_h;1;pad;act_l]
WSTK_R = 80                 # packed [wTh;0;wTh;0;wTl] for batch-major dist

f32 = mybir.dt.float32
f32r = mybir.dt.float32r
bf16 = mybir.dt.bfloat16
f16 = mybir.dt.float16
i32 = mybir.dt.int32
i16 = mybir.dt.int16
u32 = mybir.dt.uint32
Alu = mybir.AluOpType
Act = mybir.ActivationFunctionType

DEBUG = False
TRACE = False
SELECT = True

# ---------------------------------------------------------------- host helpers

def _split_f32r(x):
    """Lossless split x = hi + lo with both f32r-legal (low 12 mantissa bits zero)."""
    x = np.ascontiguousarray(x, dtype=np.float32)
    b = x.view(np.uint32)
    hi = (b & np.uint32(0xFFFFF000)).view(np.float32)
    lo = (x - hi).astype(np.float32)
    return hi, lo


def _f16_raw(x):
    return np.ascontiguousarray(np.asarray(x, dtype=np.float32).astype(np.float16))


# ---------------------------------------------------------------- kernel build

_CACHED_NC = None

def _build():
    global _CACHED_NC
    if _CACHED_NC is not None:
        return _CACHED_NC
    nc = bacc.Bacc("TRN2", target_bir_lowering=False, num_devices=NCORES)

    # ---- dram inputs ----
    ch_d = nc.dram_tensor("condh", [COND_D, NLOC], f32r, kind="ExternalInput")
    cl_d = nc.dram_tensor("condl", [COND_D, NLOC], f32r, kind="ExternalInput")
    xmix_d = nc.dram_tensor("xmix", [XMIX_R, NLOC], f32r, kind="ExternalInput")
    actl_d = nc.dram_tensor("actl", [ACT_D, NLOC], f32, kind="ExternalInput")
    w1s_d = nc.dram_tensor("w1s", [5 * 128, H1], f32r, kind="ExternalInput")
    w2h_d = nc.dram_tensor("w2h", [H1, H2], f32r, kind="ExternalInput")
    w2l_d = nc.dram_tensor("w2l", [H1, H2], f32r, kind="ExternalInput")
    w3h_d = nc.dram_tensor("w3h", [H2, E], f32r, kind="ExternalInput")
    w3l_d = nc.dram_tensor("w3l", [H2, E], f32r, kind="ExternalInput")
    b2p_d = nc.dram_tensor("b2p", [H2, 1], f32, kind="ExternalInput")
    b3p_d = nc.dram_tensor("b3p", [E, 1], f32, kind="ExternalInput")
    wTh_d = nc.dram_tensor("wTh", [E, K], f32r, kind="ExternalInput")
    wTl_d = nc.dram_tensor("wTl", [E, K], f32r, kind="ExternalInput")
    wstk_d = nc.dram_tensor("wstk", [WSTK_R, K], f32r, kind="ExternalInput")
    embT_d = nc.dram_tensor("embT", [E, K], f32, kind="ExternalInput")
    ones16_d = nc.dram_tensor("ones16", [E, 1], f32r, kind="ExternalInput")
    dw1_d = nc.dram_tensor("dw1", [COND_D + E, H2], f16, kind="ExternalInput")  # [cond; q]
    db1_d = nc.dram_tensor("db1", [H2, 1], f32, kind="ExternalInput")
    dw2_d = nc.dram_tensor("dw2", [H2, H1], f16, kind="ExternalInput")
    dw3_d = nc.dram_tensor("dw3", [H1, ACT_D], f16, kind="ExternalInput")
    db2p_d = nc.dram_tensor("db2p", [H1, 1], f32, kind="ExternalInput")
    db3p_d = nc.dram_tensor("db3p", [ACT_D, 1], f32, kind="ExternalInput")

    # ---- dram outputs ----
    recon_d = nc.dram_tensor("reconT", [ACT_D, NLOC], f32, kind="ExternalOutput")
    qT_d = nc.dram_tensor("qT", [E, NLOC], f32, kind="ExternalOutput")
    idx_d = nc.dram_tensor("idx", [NLOC], i32, kind="ExternalOutput")
    cstats_d = nc.dram_tensor("cstats", [4, K], f32, kind="ExternalOutput")
    lsums_d = nc.dram_tensor("lsums", [32], f32, kind="ExternalOutput")
    if DEBUG:
        zdbg_d = nc.dram_tensor("zdbg", [E, NLOC], f32, kind="ExternalOutput")

    # ---- internal dram ----
    dcm_d = nc.dram_tensor("dcm", [128, 2, NLOC], bf16)
    invn_d = nc.dram_tensor("invnscr", [NCHUNK, CH], f32)
    cc_in = nc.dram_tensor("cc_in", [K, 2], f32)
    cc_out = nc.dram_tensor("cc_out", [K, 2], f32, addr_space="Shared")

    with TileContext(nc) as tc:
        import contextlib
        ctx = contextlib.ExitStack()
        with ctx:
            singles = ctx.enter_context(tc.tile_pool(name="singles", bufs=1))
            loopctx = ctx.enter_context(contextlib.ExitStack())
            chunkio = loopctx.enter_context(tc.tile_pool(name="chunkio", bufs=2))
            work = loopctx.enter_context(tc.tile_pool(name="work", bufs=1))
            dbl = loopctx.enter_context(tc.tile_pool(name="dbl", bufs=2))
            small = loopctx.enter_context(tc.tile_pool(name="small", bufs=2))
            pm = loopctx.enter_context(tc.tile_pool(name="pm", bufs=3, space="PSUM"))
            psml = loopctx.enter_context(tc.tile_pool(name="psml", bufs=2, space="PSUM"))

            def load(pool, dram, shape, tag):
                t = pool.tile(shape, dram.dtype, tag=tag, name=tag)
                nc.sync.dma_start(out=t, in_=dram.ap())
                return t

            def load_kt(dram, ktiles, free, dtype, tag, rows=None):
                t = singles.tile([128, ktiles, free], dtype, tag=tag, name=tag)
                nrows = rows or dram.shape[0]
                for k_ in range(ktiles):
                    r0 = 128 * k_
                    r1 = min(r0 + 128, nrows)
                    if r1 > r0:
                        nc.sync.dma_start(out=t[0:r1 - r0, k_, :], in_=dram[r0:r1, :])
                return t

            def load_scal(dram, T, tag):
                t = singles.tile([128, T, 1], f32, tag=tag, name=tag)
                nc.sync.dma_start(out=t, in_=bass.AP(tensor=dram, offset=0,
                                                     ap=[[1, 128], [128, T], [1, 1]]))
                return t

            w1s = load_kt(w1s_d, 5, H1, f32r, "w1s")
            w2h = load_kt(w2h_d, 4, H2, f32r, "w2h")
            w2l = load_kt(w2l_d, 4, H2, f32r, "w2l")
            w3h = load_kt(w3h_d, 2, E, f32r, "w3h")
            w3l = load_kt(w3l_d, 2, E, f32r, "w3l")
            b2p = load_scal(b2p_d, 2, "b2p")
            b2p1 = singles.tile([128, 2, 1], f32, tag="b2p1", name="b2p1")
            nc.vector.tensor_scalar(out=b2p1, in0=b2p, scalar1=1.0, scalar2=None, op0=Alu.add)
            b3p = load(singles, b3p_d, [E, 1], "b3p")
            wTh = load(singles, wTh_d, [E, K], "wTh")
            wTl = load(singles, wTl_d, [E, K], "wTl")
            wstk = load(singles, wstk_d, [WSTK_R, K], "wstk")
            embT = load(singles, embT_d, [E, K], "embT")
            ones16 = load(singles, ones16_d, [E, 1], "ones16")
            dw1 = load_kt(dw1_d, 3, H2, f16, "dw1", rows=COND_D + E)
            dw2 = load_kt(dw2_d, 2, H1, f16, "dw2")
            dw3 = load_kt(dw3_d, 4, ACT_D, f16, "dw3")
            db1 = load_scal(db1_d, 2, "db1")
            db1p1 = singles.tile([128, 2, 1], f32, tag="db1p1", name="db1p1")
            nc.vector.tensor_scalar(out=db1p1, in0=db1, scalar1=1.0, scalar2=None, op0=Alu.add)
            db2p = load_scal(db2p_d, 4, "db2p")
            db2p1 = singles.tile([128, 4, 1], f32, tag="db2p1", name="db2p1")
            nc.vector.tensor_scalar(out=db2p1, in0=db2p, scalar1=1.0, scalar2=None, op0=Alu.add)
            db3p = load(singles, db3p_d, [ACT_D, 1], "db3p")

            # z-stacks for batch-major distances (double-buffered by parity)
            zstks = []
            for pz in range(2):
                zs = singles.tile([WSTK_R, CH], f32r, tag=f"zstk{pz}", name=f"zstk{pz}")
                nc.vector.memset(zs.bitcast(f32), 0.0)
                zstks.append(zs)

            qsums = singles.tile([E, NCHUNK], f32, tag="qsums", name="qsums")
            rsums = singles.tile([ACT_D, NCHUNK], f32, tag="rsums", name="rsums")

            # ================= main chunk loop =================
            for c in range(NCHUNK):
                cs0 = c * CH
                ch0 = chunkio.tile([128, CH], f32r, tag="ch0", name="ch0")
                nc.sync.dma_start(out=ch0, in_=ch_d[0:128, cs0:cs0 + CH])
                ch1 = chunkio.tile([128, CH], f32r, tag="ch1", name="ch1")
                nc.sync.dma_start(out=ch1, in_=ch_d[128:256, cs0:cs0 + CH])
                cl0 = chunkio.tile([128, CH], f32r, tag="cl0", name="cl0")
                nc.sync.dma_start(out=cl0, in_=cl_d[0:128, cs0:cs0 + CH])
                cl1 = chunkio.tile([128, CH], f32r, tag="cl1", name="cl1")
                nc.sync.dma_start(out=cl1, in_=cl_d[128:256, cs0:cs0 + CH])
                xmx = chunkio.tile([XMIX_R, CH], f32r, tag="xmx", name="xmx")
                nc.sync.dma_start(out=xmx, in_=xmix_d[:, cs0:cs0 + CH])
                actl = chunkio.tile([ACT_D, CH], f32, tag="actl", name="actl")
                nc.sync.dma_start(out=actl, in_=actl_d[:, cs0:cs0 + CH])

                # ---------- enc1 (K-stacked 3-term) ----------
                ps1 = [pm.tile([128, 2, CH], f32, tag="pm", name="ps1a"),
                       pm.tile([128, 2, CH], f32, tag="pm", name="ps1b")]
                for m in range(4):
                    pv = ps1[m // 2][:, m % 2, :]
                    ms = slice(128 * m, 128 * (m + 1))
                    terms = [(w1s[:, 0, ms], ch0), (w1s[:, 1, ms], ch1),
                             (w1s[:, 2, ms], ch0), (w1s[:, 3, ms], ch1),
                             (w1s[:, 0, ms], cl0), (w1s[:, 1, ms], cl1),
                             (w1s[0:XMIX_R, 4, ms], xmx)]
                    for i, (lw, rx) in enumerate(terms):
                        nc.tensor.matmul(pv, lw, rx, start=(i == 0), stop=(i == len(terms) - 1))

                # ---------- ELU1: p = min(exp(s), max(s+1,1)) ----------
                e1 = work.tile([128, 4, CH], f32, tag="e1", name="e1")
                r1 = work.tile([128, 4, CH], f32, tag="r1", name="r1")
                hh1 = dbl.tile([128, 4, CH], f32r, tag="hh1", name="hh1")
                hl1 = dbl.tile([128, 4, CH], f32r, tag="hl1", name="hl1")
                for half in range(2):
                    hs = slice(2 * half, 2 * half + 2)
                    nc.scalar.activation(e1[:, hs, :], ps1[half][:, :, :], Act.Exp)
                    nc.vector.tensor_scalar(out=r1[:, hs, :], in0=ps1[half][:, :, :],
                                            scalar1=1.0, scalar2=1.0, op0=Alu.add, op1=Alu.max)
                nc.vector.tensor_tensor(out=r1, in0=e1, in1=r1, op=Alu.min)
                nc.vector.tensor_copy(hh1, r1)
                nc.vector.tensor_tensor(out=hl1, in0=r1, in1=hh1.bitcast(f32), op=Alu.subtract)

                # ---------- enc2 ----------
                ps2 = pm.tile([128, 2, CH], f32, tag="pm", name="ps2")
                for m in range(2):
                    pv = ps2[:, m, :]
                    ms = slice(128 * m, 128 * (m + 1))
                    terms = []
                    for t in range(4):
                        terms.append((w2h[:, t, ms], hh1[:, t, :]))
                    for t in range(4):
                        terms.append((w2l[:, t, ms], hh1[:, t, :]))
                    for t in range(4):
                        terms.append((w2h[:, t, ms], hl1[:, t, :]))
                    for i, (lw, rx) in enumerate(terms):
                        nc.tensor.matmul(pv, lw, rx, start=(i == 0), stop=(i == len(terms) - 1))

                # ---------- ELU2 ----------
                e2 = work.tile([128, 2, CH], f32, tag="e2", name="e2")
                r2 = work.tile([128, 2, CH], f32, tag="r2", name="r2")
                hh2 = dbl.tile([128, 2, CH], f32r, tag="hh2", name="hh2")
                hl2 = dbl.tile([128, 2, CH], f32r, tag="hl2", name="hl2")
                for m in range(2):
                    nc.scalar.activation(e2[:, m, :], ps2[:, m, :], Act.Exp, bias=b2p[:, m, :])
                    nc.vector.tensor_scalar(out=r2[:, m, :], in0=ps2[:, m, :],
                                            scalar1=b2p1[:, m, :], scalar2=1.0, op0=Alu.add, op1=Alu.max)
                nc.vector.tensor_tensor(out=r2, in0=e2, in1=r2, op=Alu.min)
                nc.vector.tensor_copy(hh2, r2)
                nc.vector.tensor_tensor(out=hl2, in0=r2, in1=hh2.bitcast(f32), op=Alu.subtract)

                # ---------- enc3: z ----------
                psz = psml.tile([E, CH], f32, tag="ps_s", name="psz")
                terms = []
                for t in range(2):
                    terms.append((w3h[:, t, :], hh2[:, t, :]))
                for t in range(2):
                    terms.append((w3l[:, t, :], hh2[:, t, :]))
                for t in range(2):
                    terms.append((w3h[:, t, :], hl2[:, t, :]))
                for i, (lw, rx) in enumerate(terms):
                    nc.tensor.matmul(psz, lw, rx, start=(i == 0), stop=(i == len(terms) - 1))
                z = small.tile([E, CH], f32, tag="z", name="z")
                nc.vector.tensor_scalar(out=z, in0=psz, scalar1=b3p[:, 0:1], scalar2=None, op0=Alu.add)
                if DEBUG:
                    nc.sync.dma_start(out=zdbg_d[:, cs0:cs0 + CH], in_=z)

                # ---------- z stack (raw z hi/lo) for batch-major dist ----------
                zrh = small.tile([E, CH], f32r, tag="zrh", name="zrh")
                nc.vector.tensor_copy(zrh, z)
                zrl = small.tile([E, CH], f32r, tag="zrl", name="zrl")
                nc.vector.tensor_tensor(out=zrl, in0=z, in1=zrh.bitcast(f32), op=Alu.subtract)
                # ---------- batch-major distances + argmax ----------
                psbm = pm.tile([128, 4, 256], f32, tag="pm", name="psbm")
                for j in range(4):
                    js = slice(128 * j, 128 * (j + 1))
                    terms = [(zrh[:, js], wTh), (zrl[:, js], wTh), (zrh[:, js], wTl)]
                    for i, (lw, rx) in enumerate(terms):
                        nc.tensor.matmul(psbm[:, j, :], lw, rx, start=(i == 0), stop=(i == 2))
                m8 = small.tile([128, 4, 8], f32, tag="m8", name="m8")
                i8 = small.tile([128, 4, 8], u32, tag="i8", name="i8")
                for j in range(4):
                    nc.vector.max(m8[:, j, :], psbm[:, j, :])
                    nc.vector.max_index(i8[:, j, :], m8[:, j, :], psbm[:, j, :])
                idx32 = small.tile([128, 4], i32, tag="idx32", name="idx32")
                nc.vector.tensor_copy(idx32, i8[:, :, 0])
                nc.sync.dma_start(
                    out=bass.AP(tensor=idx_d, offset=cs0, ap=[[1, 128], [128, 4]]),
                    in_=idx32)
                idxw32 = small.tile([16, CH // 16], i32, tag="idxw32", name="idxw32")
                nc.sync.dma_start(
                    out=idxw32,
                    in_=bass.AP(tensor=idx_d, offset=cs0, ap=[[1, 16], [16, CH // 16]]))
                idxw = small.tile([16, CH // 16], i16, tag="idxw", name="idxw")
                nc.vector.tensor_copy(idxw, idxw32)

                # ---------- normalize (selection path only) ----------
                zzr = small.tile([E, CH], f32r, tag="zzr", name="zzr")
                nc.vector.tensor_tensor(out=zzr, in0=z, in1=z, op=Alu.mult)
                psn = psml.tile([1, CH], f32, tag="ps_s", name="psn")
                nc.tensor.matmul(psn, ones16, zzr, start=True, stop=True)
                nrm = small.tile([1, CH], f32, tag="nrm", name="nrm")
                nc.scalar.activation(nrm, psn, Act.Sqrt)
                invn1 = small.tile([1, CH], f32, tag="invn1", name="invn1")
                nc.vector.reciprocal_approx_fast(invn1, nrm)
                invn16 = small.tile([E, CH], f32, tag="invn16", name="invn16")
                nc.gpsimd.partition_broadcast(invn16, invn1, channels=E)
                zhat = small.tile([E, CH], f32, tag="zhat", name="zhat")
                nc.vector.tensor_tensor(out=zhat, in0=z, in1=invn16, op=Alu.mult)
                zh = small.tile([E, CH], f32r, tag="zh", name="zh")
                nc.vector.tensor_copy(zh, zhat)

                # ---------- distances code-major (selection, 1-term) ----------
                pscm = pm.tile([128, 2, CH], f32, tag="pm", name="pscm")
                for mcode in range(2):
                    nc.tensor.matmul(pscm[:, mcode, :], wTh[:, 128 * mcode:128 * (mcode + 1)], zh,
                                     start=True, stop=True)
                stg = small.tile([128, 2, CH], bf16, tag="stg", name="stg")
                nc.scalar.copy(stg, pscm)
                nc.sync.dma_start(out=dcm_d[:, :, cs0:cs0 + CH], in_=stg)

                # ---------- gather quantized ----------
                gsb = small.tile([E, CH], f32, tag="zhat", name="gsb")
                nc.gpsimd.ap_gather(gsb, embT, idxw, channels=16, num_elems=K, d=1, num_idxs=CH)
                nc.sync.dma_start(out=qT_d[:, cs0:cs0 + CH], in_=gsb)
                qd = small.tile([E, CH], f32, tag="zzr", name="qd")
                nc.vector.tensor_tensor(out=qd, in0=gsb, in1=z, op=Alu.subtract)
                nc.scalar.activation(qd, qd, Act.Square, accum_out=qsums[:, c:c + 1])

                # ---------- decoder rhs ----------
                cb0 = work.tile([128, CH], f16, tag="cb0", name="cb0")
                nc.vector.tensor_copy(cb0, ch0.bitcast(f32))
                cb1 = work.tile([128, CH], f16, tag="cb1", name="cb1")
                nc.vector.tensor_copy(cb1, ch1.bitcast(f32))
                dr2 = work.tile([16, CH], f16, tag="dr2", name="dr2")
                nc.vector.tensor_copy(dr2, gsb)

                # ---------- dec1 ----------
                psd1 = pm.tile([128, 2, CH], f32, tag="pm", name="psd1")
                for m in range(2):
                    ms = slice(128 * m, 128 * (m + 1))
                    terms = [(dw1[:, 0, ms], cb0), (dw1[:, 1, ms], cb1), (dw1[0:16, 2, ms], dr2)]
                    for i, (lw, rx) in enumerate(terms):
                        nc.tensor.matmul(psd1[:, m, :], lw, rx, start=(i == 0), stop=(i == 2))
                ed1 = work.tile([128, 2, CH], f16, tag="ed1", name="ed1")
                rd1 = work.tile([128, 2, CH], f16, tag="rd1", name="rd1")
                for m in range(2):
                    nc.scalar.activation(ed1[:, m, :], psd1[:, m, :], Act.Exp, bias=db1[:, m, :])
                    nc.vector.tensor_scalar(out=rd1[:, m, :], in0=psd1[:, m, :],
                                            scalar1=db1p1[:, m, :], scalar2=1.0,
                                            op0=Alu.add, op1=Alu.max)
                nc.vector.tensor_tensor(out=rd1, in0=ed1, in1=rd1, op=Alu.min)

                # ---------- dec2 ----------
                psd2 = [pm.tile([128, 2, CH], f32, tag="pm", name="psd2a"),
                        pm.tile([128, 2, CH], f32, tag="pm", name="psd2b")]
                for m in range(4):
                    pv = psd2[m // 2][:, m % 2, :]
                    ms = slice(128 * m, 128 * (m + 1))
                    for t in range(2):
                        nc.tensor.matmul(pv, dw2[:, t, ms], rd1[:, t, :], start=(t == 0), stop=(t == 1))
                ed2 = work.tile([128, 4, CH], f16, tag="ed2", name="ed2")
                rd2 = work.tile([128, 4, CH], f16, tag="rd2", name="rd2")
                for m in range(4):
                    nc.scalar.activation(ed2[:, m, :], psd2[m // 2][:, m % 2, :], Act.Exp,
                                         bias=db2p[:, m, :])
                    nc.vector.tensor_scalar(out=rd2[:, m, :], in0=psd2[m // 2][:, m % 2, :],
                                            scalar1=db2p1[:, m, :], scalar2=1.0,
                                            op0=Alu.add, op1=Alu.max)
                nc.vector.tensor_tensor(out=rd2, in0=ed2, in1=rd2, op=Alu.min)

                # ---------- dec3 ----------
                psd3 = psml.tile([ACT_D, CH], f32, tag="ps_s", name="psd3")
                for t in range(4):
                    nc.tensor.matmul(psd3, dw3[:, t, 0:ACT_D], rd2[:, t, :], start=(t == 0), stop=(t == 3))
                recon = small.tile([ACT_D, CH], f32, tag="recon", name="recon")
                nc.vector.tensor_scalar(out=recon, in0=psd3, scalar1=db3p[:, 0:1], scalar2=None, op0=Alu.add)
                nc.sync.dma_start(out=recon_d[:, cs0:cs0 + CH], in_=recon)
                rdf = small.tile([ACT_D, CH], f32, tag="z", name="rdf")
                nc.vector.tensor_tensor(out=rdf, in0=recon, in1=xmx[0:12, :].bitcast(f32), op=Alu.subtract)
                nc.vector.tensor_tensor(out=rdf, in0=rdf, in1=actl, op=Alu.subtract)
                nc.scalar.activation(rdf, rdf, Act.Square, accum_out=rsums[:, c:c + 1])

            # ================= selection (STUBBED for bisection) =================
            loopctx.close()
            sel = ctx.enter_context(tc.tile_pool(name="sel", bufs=1))
            zz_ = sel.tile([128, 2], f32, tag="zz_", name="zz_")
            nc.vector.memset(zz_, 0.0)
            for r in range(4):
                nc.sync.dma_start(
                    out=bass.AP(tensor=cstats_d, offset=r * K, ap=[[1, 128], [128, 2]]),
                    in_=zz_)
            qtot = sel.tile([E, 1], f32, tag="qtot", name="qtot")
            nc.vector.tensor_reduce(out=qtot, in_=qsums, axis=mybir.AxisListType.X, op=Alu.add)
            rtot = sel.tile([ACT_D, 1], f32, tag="rtot", name="rtot")
            nc.vector.tensor_reduce(out=rtot, in_=rsums, axis=mybir.AxisListType.X, op=Alu.add)
            nc.sync.dma_start(out=bass.AP(tensor=lsums_d, offset=0, ap=[[1, E]]), in_=qtot)
            nc.sync.dma_start(out=bass.AP(tensor=lsums_d, offset=16, ap=[[1, ACT_D]]), in_=rtot)

    nc.compile()
    _CACHED_NC = nc
    return nc


# ---------------------------------------------------------------- host wrapper

def kernel(actions, conditions, enc_w1, enc_b1, enc_w2, enc_b2, enc_w3, enc_b3,
           dec_w1, dec_b1, dec_w2, dec_b2, dec_w3, dec_b3, embedding):
    actions = np.asarray(actions, dtype=np.float32)
    conditions = np.asarray(conditions, dtype=np.float32)
    enc_w1 = np.asarray(enc_w1, dtype=np.float32)
    enc_b1 = np.asarray(enc_b1, dtype=np.float32)
    enc_w2 = np.asarray(enc_w2, dtype=np.float32)
    enc_b2 = np.asarray(enc_b2, dtype=np.float32)
    enc_w3 = np.asarray(enc_w3, dtype=np.float32)
    enc_b3 = np.asarray(enc_b3, dtype=np.float32)
    dec_w1 = np.asarray(dec_w1, dtype=np.float32)
    dec_b1 = np.asarray(dec_b1, dtype=np.float32)
    dec_w2 = np.asarray(dec_w2, dtype=np.float32)
    dec_b2 = np.asarray(dec_b2, dtype=np.float32)
    dec_w3 = np.asarray(dec_w3, dtype=np.float32)
    dec_b3 = np.asarray(dec_b3, dtype=np.float32)
    embedding = np.asarray(embedding, dtype=np.float32)

    # ---- weight prep ----
    Wc = enc_w1[ACT_D:, :]
    Wa = enc_w1[:ACT_D, :]
    Wch, Wcl = _split_f32r(Wc)
    Wah, Wal = _split_f32r(Wa)
    b1h, b1l = _split_f32r(enc_b1[None, :])
    w1s = np.zeros((5 * 128, H1), np.float32)
    w1s[0:128] = Wch[0:128]
    w1s[128:256] = Wch[128:256]
    w1s[256:384] = Wcl[0:128]
    w1s[384:512] = Wcl[128:256]
    w1s[512:524] = Wah
    w1s[524:525] = b1h
    w1s[525:537] = Wal
    w1s[537:538] = b1l
    w1s[544:556] = Wah
    w2h, w2l = _split_f32r(enc_w2)
    w3h, w3l = _split_f32r(enc_w3)
    b2p = (enc_b2.astype(np.float64) - enc_w2.astype(np.float64).sum(0)).astype(np.float32)[:, None]
    b3p = (enc_b3.astype(np.float64) - enc_w3.astype(np.float64).sum(0)).astype(np.float32)[:, None]
    wn = embedding / np.maximum(np.linalg.norm(embedding, axis=1, keepdims=True), 1e-12)
    wTh, wTl = _split_f32r(np.ascontiguousarray(wn.T))
    wstk = np.zeros((WSTK_R, K), np.float32)
    wstk[0:16] = wTh
    wstk[32:48] = wTh
    wstk[64:80] = wTl
    embT = np.ascontiguousarray(embedding.T)
    ones16 = np.ones((E, 1), np.float32)
    DW1 = np.concatenate([dec_w1[E:, :], dec_w1[:E, :]], 0)
    dw1 = _f16_raw(DW1)
    dw2 = _f16_raw(dec_w2)
    dw3 = _f16_raw(dec_w3)
    db2p = (dec_b2.astype(np.float64) - dec_w2.astype(np.float16).astype(np.float64).sum(0)).astype(np.float32)[:, None]
    db3p = (dec_b3.astype(np.float64) - dec_w3.astype(np.float16).astype(np.float64).sum(0)).astype(np.float32)[:, None]

    shared = dict(w1s=w1s, w2h=w2h, w2l=w2l, w3h=w3h, w3l=w3l,
                  b2p=b2p, b3p=b3p, wTh=wTh, wTl=wTl, wstk=wstk, embT=embT, ones16=ones16,
                  dw1=dw1, dw2=dw2, dw3=dw3, db1=dec_b1[:, None].astype(np.float32),
                  db2p=db2p, db3p=db3p)

    in_maps = []
    for i in range(NCORES):
        sl = slice(i * NLOC, (i + 1) * NLOC)
        condT = np.ascontiguousarray(conditions[sl].T)
        chh, cll = _split_f32r(condT)
        actT = np.ascontiguousarray(actions[sl].T)
        ah, al = _split_f32r(actT)
        xmix = np.zeros((XMIX_R, NLOC), np.float32)
        xmix[0:12] = ah
        xmix[12] = 1.0
        xmix[13:25] = ah
        xmix[25] = 1.0
        xmix[32:44] = al
        m = dict(shared)
        m["condh"] = chh
        m["condl"] = cll
        m["xmix"] = xmix
        m["actl"] = np.ascontiguousarray(al)
        in_maps.append(m)

    nc = _build()
    res = run_bass_kernel_spmd(nc, in_maps, core_ids=list(range(NCORES)), trace=TRACE)
    results = res.results
    kernel._last_exec_time_ns = res.exec_time_ns
    kernel._last_results = results

    # ---- host unshard / finish ----
    reconstructed = np.concatenate([r["reconT"].T for r in results], 0)
    quantized_st = np.concatenate([r["qT"].T for r in results], 0)
    idx = np.concatenate([r["idx"] for r in results], 0).astype(np.int32)

    qsum = np.sum([r["lsums"][0:16] for r in results], axis=(0, 1))
    rsum = np.sum([r["lsums"][16:28] for r in results], axis=(0, 1))
    q_latent = np.float32(qsum / (N * E))
    e_latent = np.float32(COMMIT * (qsum / (N * E)))
    recon_loss = np.float32(rsum / (N * ACT_D))

    counts = np.bincount(idx, minlength=K).astype(np.float64)
    avg = counts / N
    perplexity = np.float32(np.exp(-np.sum(avg * np.log(avg + 1e-10))))

    A_hi = np.sum([r["cstats"][0] for r in results], 0).astype(np.float64)
    A_lo = np.sum([r["cstats"][1] for r in results], 0).astype(np.float64)
    t_hi = results[0]["cstats"][2].astype(np.float64)
    t_lo = results[0]["cstats"][3].astype(np.float64)
    n_pos = N // K
    pos = (A_hi - (N - n_pos) * t_hi) / n_pos
    S = A_lo - N / 2.0
    lse = t_lo / TAU + np.log(np.exp((pos - t_lo) / TAU) + S)
    contra = np.float32(np.mean(lse - pos / TAU))

    return (reconstructed, quantized_st, idx, q_latent, e_latent, contra,
            np.float32(perplexity), recon_loss)


kernel._last_exec_time_ns = None
kernel._last_results = None


# revision 25
# speedup vs baseline: 1.2416x; 1.0272x over previous
"""Trainium2 Bass kernel for nn_ActionQuantizer (vq_codebook).

Self-contained: hardcodes shapes/sharding. Accepts FULL inputs, returns FULL outputs.
Data-parallel over batch N across 8 cores; weights/codebook replicated.

v2: K-stacked f32r 3-term matmuls, DRAM-resident selection matrix, deeper buffering.
"""
import sys
sys.path.insert(0, "/opt/trn_rl_repo")

import numpy as np
import concourse.bass as bass
from concourse import bacc
import concourse.mybir as mybir
from concourse.tile import TileContext
from concourse.bass_utils import run_bass_kernel_spmd

# ---- problem constants ----
N = 131072
ACT_D = 12
COND_D = 256
K = 256            # codebook size
E = 16             # embedding dim
H1, H2 = 512, 256
COMMIT = 0.25
TAU = 0.07

NCORES = 8
NLOC = N // NCORES          # 16384
CH = 512                    # batch chunk (moving dim)
NCHUNK = NLOC // CH         # 32
ASUB = 4096                 # phase-A subsample columns
NEWTON_ITERS = 4
XMIX_R = 44                 # packed [act_h;1;act_h;1;pad;act_l]
WSTK_R = 80                 # packed [wTh;0;wTh;0;wTl] for batch-major dist

f32 = mybir.dt.float32
f32r = mybir.dt.float32r
bf16 = mybir.dt.bfloat16
f16 = mybir.dt.float16
i32 = mybir.dt.int32
i16 = mybir.dt.int16
u32 = mybir.dt.uint32
Alu = mybir.AluOpType
Act = mybir.ActivationFunctionType

DEBUG = False
TRACE = False
SELECT = True

# ---------------------------------------------------------------- host helpers

def _split_f32r(x):
    """Lossless split x = hi + lo with both f32r-legal (low 12 mantissa bits zero)."""
    x = np.ascontiguousarray(x, dtype=np.float32)
    b = x.view(np.uint32)
    hi = (b & np.uint32(0xFFFFF000)).view(np.float32)
    lo = (x - hi).astype(np.float32)
    return hi, lo


def _f16_raw(x):
    return np.ascontiguousarray(np.asarray(x, dtype=np.float32).astype(np.float16))


# ---------------------------------------------------------------- kernel build

_CACHED_NC = None

def _build():
    global _CACHED_NC
    if _CACHED_NC is not None:
        return _CACHED_NC
    nc = bacc.Bacc("TRN2", target_bir_lowering=False, num_devices=NCORES)

    # ---- dram inputs ----
    ch_d = nc.dram_tensor("condh", [COND_D, NLOC], f32r, kind="ExternalInput")
    cl_d = nc.dram_tensor("condl", [COND_D, NLOC], f32r, kind="ExternalInput")
    xmix_d = nc.dram_tensor("xmix", [XMIX_R, NLOC], f32r, kind="ExternalInput")
    actl_d = nc.dram_tensor("actl", [ACT_D, NLOC], f32, kind="ExternalInput")
    w1s_d = nc.dram_tensor("w1s", [5 * 128, H1], f32r, kind="ExternalInput")
    w2h_d = nc.dram_tensor("w2h", [H1, H2], f32r, kind="ExternalInput")
    w2l_d = nc.dram_tensor("w2l", [H1, H2], f32r, kind="ExternalInput")
    w3h_d = nc.dram_tensor("w3h", [H2, E], f32r, kind="ExternalInput")
    w3l_d = nc.dram_tensor("w3l", [H2, E], f32r, kind="ExternalInput")
    b2p_d = nc.dram_tensor("b2p", [H2, 1], f32, kind="ExternalInput")
    b3p_d = nc.dram_tensor("b3p", [E, 1], f32, kind="ExternalInput")
    wTh_d = nc.dram_tensor("wTh", [E, K], f32r, kind="ExternalInput")
    wTl_d = nc.dram_tensor("wTl", [E, K], f32r, kind="ExternalInput")
    wstk_d = nc.dram_tensor("wstk", [WSTK_R, K], f32r, kind="ExternalInput")
    embT_d = nc.dram_tensor("embT", [E, K], f32, kind="ExternalInput")
    ones16_d = nc.dram_tensor("ones16", [E, 1], f32r, kind="ExternalInput")
    dw1_d = nc.dram_tensor("dw1", [COND_D + E, H2], f16, kind="ExternalInput")  # [cond; q]
    db1_d = nc.dram_tensor("db1", [H2, 1], f32, kind="ExternalInput")
    dw2_d = nc.dram_tensor("dw2", [H2, H1], f16, kind="ExternalInput")
    dw3_d = nc.dram_tensor("dw3", [H1, ACT_D], f16, kind="ExternalInput")
    db2p_d = nc.dram_tensor("db2p", [H1, 1], f32, kind="ExternalInput")
    db3p_d = nc.dram_tensor("db3p", [ACT_D, 1], f32, kind="ExternalInput")

    # ---- dram outputs ----
    recon_d = nc.dram_tensor("reconT", [ACT_D, NLOC], f32, kind="ExternalOutput")
    qT_d = nc.dram_tensor("qT", [E, NLOC], f32, kind="ExternalOutput")
    idx_d = nc.dram_tensor("idx", [NLOC], i32, kind="ExternalOutput")
    cstats_d = nc.dram_tensor("cstats", [4, K], f32, kind="ExternalOutput")
    lsums_d = nc.dram_tensor("lsums", [32], f32, kind="ExternalOutput")
    if DEBUG:
        zdbg_d = nc.dram_tensor("zdbg", [E, NLOC], f32, kind="ExternalOutput")

    # ---- internal dram ----
    dcm_d = nc.dram_tensor("dcm", [128, 2, NLOC], bf16)
    invn_d = nc.dram_tensor("invnscr", [NCHUNK, CH], f32)
    cc_in = nc.dram_tensor("cc_in", [K, 2], f32)
    cc_out = nc.dram_tensor("cc_out", [K, 2], f32, addr_space="Shared")

    with TileContext(nc) as tc:
        import contextlib
        ctx = contextlib.ExitStack()
        with ctx:
            singles = ctx.enter_context(tc.tile_pool(name="singles", bufs=1))
            loopctx = ctx.enter_context(contextlib.ExitStack())
            chunkio = loopctx.enter_context(tc.tile_pool(name="chunkio", bufs=2))
            work = loopctx.enter_context(tc.tile_pool(name="work", bufs=1))
            dbl = loopctx.enter_context(tc.tile_pool(name="dbl", bufs=2))
            small = loopctx.enter_context(tc.tile_pool(name="small", bufs=2))
            pm = loopctx.enter_context(tc.tile_pool(name="pm", bufs=3, space="PSUM"))
            psml = loopctx.enter_context(tc.tile_pool(name="psml", bufs=2, space="PSUM"))

            def load(pool, dram, shape, tag):
                t = pool.tile(shape, dram.dtype, tag=tag, name=tag)
                nc.sync.dma_start(out=t, in_=dram.ap())
                return t

            def load_kt(dram, ktiles, free, dtype, tag, rows=None):
                t = singles.tile([128, ktiles, free], dtype, tag=tag, name=tag)
                nrows = rows or dram.shape[0]
                for k_ in range(ktiles):
                    r0 = 128 * k_
                    r1 = min(r0 + 128, nrows)
                    if r1 > r0:
                        nc.sync.dma_start(out=t[0:r1 - r0, k_, :], in_=dram[r0:r1, :])
                return t

            def load_scal(dram, T, tag):
                t = singles.tile([128, T, 1], f32, tag=tag, name=tag)
                nc.sync.dma_start(out=t, in_=bass.AP(tensor=dram, offset=0,
                                                     ap=[[1, 128], [128, T], [1, 1]]))
                return t

            w1s = load_kt(w1s_d, 5, H1, f32r, "w1s")
            w2h = load_kt(w2h_d, 4, H2, f32r, "w2h")
            w2l = load_kt(w2l_d, 4, H2, f32r, "w2l")
            w3h = load_kt(w3h_d, 2, E, f32r, "w3h")
            w3l = load_kt(w3l_d, 2, E, f32r, "w3l")
            b2p = load_scal(b2p_d, 2, "b2p")
            b2p1 = singles.tile([128, 2, 1], f32, tag="b2p1", name="b2p1")
            nc.vector.tensor_scalar(out=b2p1, in0=b2p, scalar1=1.0, scalar2=None, op0=Alu.add)
            b3p = load(singles, b3p_d, [E, 1], "b3p")
            wTh = load(singles, wTh_d, [E, K], "wTh")
            wTl = load(singles, wTl_d, [E, K], "wTl")
            wstk = load(singles, wstk_d, [WSTK_R, K], "wstk")
            embT = load(singles, embT_d, [E, K], "embT")
            ones16 = load(singles, ones16_d, [E, 1], "ones16")
            dw1 = load_kt(dw1_d, 3, H2, f16, "dw1", rows=COND_D + E)
            dw2 = load_kt(dw2_d, 2, H1, f16, "dw2")
            dw3 = load_kt(dw3_d, 4, ACT_D, f16, "dw3")
            db1 = load_scal(db1_d, 2, "db1")
            db1p1 = singles.tile([128, 2, 1], f32, tag="db1p1", name="db1p1")
            nc.vector.tensor_scalar(out=db1p1, in0=db1, scalar1=1.0, scalar2=None, op0=Alu.add)
            db2p = load_scal(db2p_d, 4, "db2p")
            db2p1 = singles.tile([128, 4, 1], f32, tag="db2p1", name="db2p1")
            nc.vector.tensor_scalar(out=db2p1, in0=db2p, scalar1=1.0, scalar2=None, op0=Alu.add)
            db3p = load(singles, db3p_d, [ACT_D, 1], "db3p")

            # z-stacks for batch-major distances (double-buffered by parity)
            zstks = []
            for pz in range(2):
                zs = singles.tile([WSTK_R, CH], f32r, tag=f"zstk{pz}", name=f"zstk{pz}")
                nc.vector.memset(zs.bitcast(f32), 0.0)
                zstks.append(zs)

            qsums = singles.tile([E, NCHUNK], f32, tag="qsums", name="qsums")
            rsums = singles.tile([ACT_D, NCHUNK], f32, tag="rsums", name="rsums")

            # ================= main chunk loop =================
            for c in range(NCHUNK):
                cs0 = c * CH
                ch0 = chunkio.tile([128, CH], f32r, tag="ch0", name="ch0")
                nc.sync.dma_start(out=ch0, in_=ch_d[0:128, cs0:cs0 + CH])
                ch1 = chunkio.tile([128, CH], f32r, tag="ch1", name="ch1")
                nc.sync.dma_start(out=ch1, in_=ch_d[128:256, cs0:cs0 + CH])
                cl0 = chunkio.tile([128, CH], f32r, tag="cl0", name="cl0")
                nc.sync.dma_start(out=cl0, in_=cl_d[0:128, cs0:cs0 + CH])
                cl1 = chunkio.tile([128, CH], f32r, tag="cl1", name="cl1")
                nc.sync.dma_start(out=cl1, in_=cl_d[128:256, cs0:cs0 + CH])
                xmx = chunkio.tile([XMIX_R, CH], f32r, tag="xmx", name="xmx")
                nc.sync.dma_start(out=xmx, in_=xmix_d[:, cs0:cs0 + CH])
                actl = chunkio.tile([ACT_D, CH], f32, tag="actl", name="actl")
                nc.sync.dma_start(out=actl, in_=actl_d[:, cs0:cs0 + CH])

                # ---------- enc1 (K-stacked 3-term) ----------
                ps1 = [pm.tile([128, 2, CH], f32, tag="pm", name="ps1a"),
                       pm.tile([128, 2, CH], f32, tag="pm", name="ps1b")]
                for m in range(4):
                    pv = ps1[m // 2][:, m % 2, :]
                    ms = slice(128 * m, 128 * (m + 1))
                    terms = [(w1s[:, 0, ms], ch0), (w1s[:, 1, ms], ch1),
                             (w1s[:, 2, ms], ch0), (w1s[:, 3, ms], ch1),
                             (w1s[:, 0, ms], cl0), (w1s[:, 1, ms], cl1),
                             (w1s[0:XMIX_R, 4, ms], xmx)]
                    for i, (lw, rx) in enumerate(terms):
                        nc.tensor.matmul(pv, lw, rx, start=(i == 0), stop=(i == len(terms) - 1))

                # ---------- ELU1: p = min(exp(s), max(s+1,1)) ----------
                e1 = work.tile([128, 4, CH], f32, tag="e1", name="e1")
                r1 = work.tile([128, 4, CH], f32, tag="r1", name="r1")
                hh1 = dbl.tile([128, 4, CH], f32r, tag="hh1", name="hh1")
                hl1 = dbl.tile([128, 4, CH], f32r, tag="hl1", name="hl1")
                for half in range(2):
                    hs = slice(2 * half, 2 * half + 2)
                    nc.scalar.activation(e1[:, hs, :], ps1[half][:, :, :], Act.Exp)
                    nc.vector.tensor_scalar(out=r1[:, hs, :], in0=ps1[half][:, :, :],
                                            scalar1=1.0, scalar2=1.0, op0=Alu.add, op1=Alu.max)
                nc.vector.tensor_tensor(out=r1, in0=e1, in1=r1, op=Alu.min)
                nc.vector.tensor_copy(hh1, r1)
                nc.vector.tensor_tensor(out=hl1, in0=r1, in1=hh1.bitcast(f32), op=Alu.subtract)

                # ---------- enc2 ----------
                ps2 = pm.tile([128, 2, CH], f32, tag="pm", name="ps2")
                for m in range(2):
                    pv = ps2[:, m, :]
                    ms = slice(128 * m, 128 * (m + 1))
                    terms = []
                    for t in range(4):
                        terms.append((w2h[:, t, ms], hh1[:, t, :]))
                    for t in range(4):
                        terms.append((w2l[:, t, ms], hh1[:, t, :]))
                    for t in range(4):
                        terms.append((w2h[:, t, ms], hl1[:, t, :]))
                    for i, (lw, rx) in enumerate(terms):
                        nc.tensor.matmul(pv, lw, rx, start=(i == 0), stop=(i == len(terms) - 1))

                # ---------- ELU2 ----------
                e2 = work.tile([128, 2, CH], f32, tag="e2", name="e2")
                r2 = work.tile([128, 2, CH], f32, tag="r2", name="r2")
                hh2 = dbl.tile([128, 2, CH], f32r, tag="hh2", name="hh2")
                hl2 = dbl.tile([128, 2, CH], f32r, tag="hl2", name="hl2")
                for m in range(2):
                    nc.scalar.activation(e2[:, m, :], ps2[:, m, :], Act.Exp, bias=b2p[:, m, :])
                    nc.vector.tensor_scalar(out=r2[:, m, :], in0=ps2[:, m, :],
                                            scalar1=b2p1[:, m, :], scalar2=1.0, op0=Alu.add, op1=Alu.max)
                nc.vector.tensor_tensor(out=r2, in0=e2, in1=r2, op=Alu.min)
                nc.vector.tensor_copy(hh2, r2)
                nc.vector.tensor_tensor(out=hl2, in0=r2, in1=hh2.bitcast(f32), op=Alu.subtract)

                # ---------- enc3: z ----------
                psz = psml.tile([E, CH], f32, tag="ps_s", name="psz")
                terms = []
                for t in range(2):
                    terms.append((w3h[:, t, :], hh2[:, t, :]))
                for t in range(2):
                    terms.append((w3l[:, t, :], hh2[:, t, :]))
                for t in range(2):
                    terms.append((w3h[:, t, :], hl2[:, t, :]))
                for i, (lw, rx) in enumerate(terms):
                    nc.tensor.matmul(psz, lw, rx, start=(i == 0), stop=(i == len(terms) - 1))
                z = small.tile([E, CH], f32, tag="z", name="z")
                nc.vector.tensor_scalar(out=z, in0=psz, scalar1=b3p[:, 0:1], scalar2=None, op0=Alu.add)
                if DEBUG:
                    nc.sync.dma_start(out=zdbg_d[:, cs0:cs0 + CH], in_=z)

                # ---------- z stack (raw z hi/lo) for batch-major dist ----------
                zrh = small.tile([E, CH], f32r, tag="zrh", name="zrh")
                nc.vector.tensor_copy(zrh, z)
                zrl = small.tile([E, CH], f32r, tag="zrl", name="zrl")
                nc.vector.tensor_tensor(out=zrl, in0=z, in1=zrh.bitcast(f32), op=Alu.subtract)
                # ---------- batch-major distances + argmax ----------
                psbm = pm.tile([128, 4, 256], f32, tag="pm", name="psbm")
                for j in range(4):
                    js = slice(128 * j, 128 * (j + 1))
                    terms = [(zrh[:, js], wTh), (zrl[:, js], wTh), (zrh[:, js], wTl)]
                    for i, (lw, rx) in enumerate(terms):
                        nc.tensor.matmul(psbm[:, j, :], lw, rx, start=(i == 0), stop=(i == 2))
                m8 = small.tile([128, 4, 8], f32, tag="m8", name="m8")
                i8 = small.tile([128, 4, 8], u32, tag="i8", name="i8")
                for j in range(4):
                    nc.vector.max(m8[:, j, :], psbm[:, j, :])
                    nc.vector.max_index(i8[:, j, :], m8[:, j, :], psbm[:, j, :])
                idx32 = small.tile([128, 4], i32, tag="idx32", name="idx32")
                nc.vector.tensor_copy(idx32, i8[:, :, 0])
                nc.sync.dma_start(
                    out=bass.AP(tensor=idx_d, offset=cs0, ap=[[1, 128], [128, 4]]),
                    in_=idx32)
                idxw32 = small.tile([16, CH // 16], i32, tag="idxw32", name="idxw32")
                nc.sync.dma_start(
                    out=idxw32,
                    in_=bass.AP(tensor=idx_d, offset=cs0, ap=[[1, 16], [16, CH // 16]]))
                idxw = small.tile([16, CH // 16], i16, tag="idxw", name="idxw")
                nc.vector.tensor_copy(idxw, idxw32)

                # ---------- normalize (selection path only) ----------
                zzr = small.tile([E, CH], f32r, tag="zzr", name="zzr")
                nc.vector.tensor_tensor(out=zzr, in0=z, in1=z, op=Alu.mult)
                psn = psml.tile([1, CH], f32, tag="ps_s", name="psn")
                nc.tensor.matmul(psn, ones16, zzr, start=True, stop=True)
                nrm = small.tile([1, CH], f32, tag="nrm", name="nrm")
                nc.scalar.activation(nrm, psn, Act.Sqrt)
                invn1 = small.tile([1, CH], f32, tag="invn1", name="invn1")
                nc.vector.reciprocal_approx_fast(invn1, nrm)
                invn16 = small.tile([E, CH], f32, tag="invn16", name="invn16")
                nc.gpsimd.partition_broadcast(invn16, invn1, channels=E)
                zhat = small.tile([E, CH], f32, tag="zhat", name="zhat")
                nc.vector.tensor_tensor(out=zhat, in0=z, in1=invn16, op=Alu.mult)
                zh = small.tile([E, CH], f32r, tag="zh", name="zh")
                nc.vector.tensor_copy(zh, zhat)

                # ---------- distances code-major (selection, 1-term) ----------
                pscm = pm.tile([128, 2, CH], f32, tag="pm", name="pscm")
                for mcode in range(2):
                    nc.tensor.matmul(pscm[:, mcode, :], wTh[:, 128 * mcode:128 * (mcode + 1)], zh,
                                     start=True, stop=True)
                stg = small.tile([128, 2, CH], bf16, tag="stg", name="stg")
                nc.scalar.copy(stg, pscm)
                nc.sync.dma_start(out=dcm_d[:, :, cs0:cs0 + CH], in_=stg)

                # ---------- gather quantized ----------
                gsb = small.tile([E, CH], f32, tag="zhat", name="gsb")
                nc.gpsimd.ap_gather(gsb, embT, idxw, channels=16, num_elems=K, d=1, num_idxs=CH)
                nc.sync.dma_start(out=qT_d[:, cs0:cs0 + CH], in_=gsb)
                qd = small.tile([E, CH], f32, tag="zzr", name="qd")
                nc.vector.tensor_tensor(out=qd, in0=gsb, in1=z, op=Alu.subtract)
                nc.scalar.activation(qd, qd, Act.Square, accum_out=qsums[:, c:c + 1])

                # ---------- decoder rhs ----------
                cb0 = work.tile([128, CH], f16, tag="cb0", name="cb0")
                nc.vector.tensor_copy(cb0, ch0.bitcast(f32))
                cb1 = work.tile([128, CH], f16, tag="cb1", name="cb1")
                nc.vector.tensor_copy(cb1, ch1.bitcast(f32))
                dr2 = work.tile([16, CH], f16, tag="dr2", name="dr2")
                nc.vector.tensor_copy(dr2, gsb)

                # ---------- dec1 ----------
                psd1 = pm.tile([128, 2, CH], f32, tag="pm", name="psd1")
                for m in range(2):
                    ms = slice(128 * m, 128 * (m + 1))
                    terms = [(dw1[:, 0, ms], cb0), (dw1[:, 1, ms], cb1), (dw1[0:16, 2, ms], dr2)]
                    for i, (lw, rx) in enumerate(terms):
                        nc.tensor.matmul(psd1[:, m, :], lw, rx, start=(i == 0), stop=(i == 2))
                ed1 = work.tile([128, 2, CH], f16, tag="ed1", name="ed1")
                rd1 = work.tile([128, 2, CH], f16, tag="rd1", name="rd1")
                for m in range(2):
                    nc.scalar.activation(ed1[:, m, :], psd1[:, m, :], Act.Exp, bias=db1[:, m, :])
                    nc.vector.tensor_scalar(out=rd1[:, m, :], in0=psd1[:, m, :],
                                            scalar1=db1p1[:, m, :], scalar2=1.0,
                                            op0=Alu.add, op1=Alu.max)
                nc.vector.tensor_tensor(out=rd1, in0=ed1, in1=rd1, op=Alu.min)

                # ---------- dec2 ----------
                psd2 = [pm.tile([128, 2, CH], f32, tag="pm", name="psd2a"),
                        pm.tile([128, 2, CH], f32, tag="pm", name="psd2b")]
                for m in range(4):
                    pv = psd2[m // 2][:, m % 2, :]
                    ms = slice(128 * m, 128 * (m + 1))
                    for t in range(2):
                        nc.tensor.matmul(pv, dw2[:, t, ms], rd1[:, t, :], start=(t == 0), stop=(t == 1))
                ed2 = work.tile([128, 4, CH], f16, tag="ed2", name="ed2")
                rd2 = work.tile([128, 4, CH], f16, tag="rd2", name="rd2")
                for m in range(4):
                    nc.scalar.activation(ed2[:, m, :], psd2[m // 2][:, m % 2, :], Act.Exp,
                                         bias=db2p[:, m, :])
                    nc.vector.tensor_scalar(out=rd2[:, m, :], in0=psd2[m // 2][:, m % 2, :],
                                            scalar1=db2p1[:, m, :], scalar2=1.0,
                                            op0=Alu.add, op1=Alu.max)
                nc.vector.tensor_tensor(out=rd2, in0=ed2, in1=rd2, op=Alu.min)

                # ---------- dec3 ----------
                psd3 = psml.tile([ACT_D, CH], f32, tag="ps_s", name="psd3")
                for t in range(4):
                    nc.tensor.matmul(psd3, dw3[:, t, 0:ACT_D], rd2[:, t, :], start=(t == 0), stop=(t == 3))
                recon = small.tile([ACT_D, CH], f32, tag="recon", name="recon")
                nc.vector.tensor_scalar(out=recon, in0=psd3, scalar1=db3p[:, 0:1], scalar2=None, op0=Alu.add)
                nc.sync.dma_start(out=recon_d[:, cs0:cs0 + CH], in_=recon)
                rdf = small.tile([ACT_D, CH], f32, tag="z", name="rdf")
                nc.vector.tensor_tensor(out=rdf, in0=recon, in1=xmx[0:12, :].bitcast(f32), op=Alu.subtract)
                nc.vector.tensor_tensor(out=rdf, in0=rdf, in1=actl, op=Alu.subtract)
                nc.scalar.activation(rdf, rdf, Act.Square, accum_out=rsums[:, c:c + 1])

            # ================= selection =================
            loopctx.close()
            if not SELECT:
                _CACHED = None
            sel = ctx.enter_context(tc.tile_pool(name="sel", bufs=1))
            selps = ctx.enter_context(tc.tile_pool(name="selps", bufs=2))

            # ---- phase A on the first ASUB columns (from DRAM) ----
            dA = sel.tile([128, 2, ASUB], bf16, tag="dA", name="dA")
            nc.sync.dma_start(out=dA, in_=dcm_d[:, :, 0:ASUB])

            s1 = sel.tile([128, 2], f32, tag="s1", name="s1")
            s2 = sel.tile([128, 2], f32, tag="s2", name="s2")
            s2c = sel.tile([128, 2, 2], f32, tag="s2c", name="s2c")
            for kt in range(2):
                dummy = selps.tile([128, ASUB], bf16, tag="scr", name="dummy")
                nc.vector.tensor_scalar(out=dummy, in0=dA[:, kt, :], scalar1=0.0, scalar2=0.0,
                                        op0=Alu.add, op1=Alu.add, accum_out=s1[:, kt:kt + 1])
                for h in range(2):
                    sq = selps.tile([128, ASUB // 2], f32, tag="scr", name="sq")
                    hs = slice(h * (ASUB // 2), (h + 1) * (ASUB // 2))
                    nc.scalar.activation(sq, dA[:, kt, hs], Act.Square,
                                         accum_out=s2c[:, kt, h:h + 1])
            nc.vector.tensor_reduce(out=s2, in_=s2c, axis=mybir.AxisListType.X, op=Alu.add)
            mu = sel.tile([128, 2], f32, tag="mu", name="mu")
            nc.vector.tensor_scalar(out=mu, in0=s1, scalar1=1.0 / ASUB, scalar2=None, op0=Alu.mult)
            var = sel.tile([128, 2], f32, tag="var", name="var")
            nc.vector.tensor_scalar(out=var, in0=s2, scalar1=1.0 / ASUB, scalar2=None, op0=Alu.mult)
            mumu = sel.tile([128, 2], f32, tag="mumu", name="mumu")
            nc.vector.tensor_tensor(out=mumu, in0=mu, in1=mu, op=Alu.mult)
            nc.vector.tensor_tensor(out=var, in0=var, in1=mumu, op=Alu.subtract)
            sd = sel.tile([128, 2], f32, tag="sd", name="sd")
            nc.scalar.activation(sd, var, Act.Sqrt)
            invsd = sel.tile([128, 2], f32, tag="invsd", name="invsd")
            nc.vector.reciprocal_approx_fast(invsd, sd)
            sdh = sel.tile([128, 2], f32, tag="sdh", name="sdh")
            nc.vector.tensor_scalar(out=sdh, in0=sd, scalar1=0.5, scalar2=None, op0=Alu.mult)
            sdhn = sel.tile([128, 2], f32, tag="sdhn", name="sdhn")
            nc.vector.tensor_scalar(out=sdhn, in0=sdh, scalar1=-1.0, scalar2=None, op0=Alu.mult)

            th = sel.tile([128, 2], f32, tag="th", name="th")
            tsc = sel.tile([128, 2], f32, tag="tsc", name="tsc")
            nc.vector.tensor_scalar(out=tsc, in0=sd, scalar1=2.653, scalar2=None, op0=Alu.mult)
            nc.vector.tensor_tensor(out=th, in0=mu, in1=tsc, op=Alu.add)
            tl = sel.tile([128, 2], f32, tag="tl", name="tl")
            nc.vector.tensor_copy(tl, mu)

            cnt = sel.tile([128, 2], f32, tag="cnt", name="cnt")
            targ_hi = ASUB * 512.0 / N
            targ_lo = ASUB * 0.5
            for it in range(NEWTON_ITERS):
                for (tt_, cmp_op, targ) in [(th, Alu.is_gt, targ_hi), (tl, Alu.is_lt, targ_lo)]:
                    for kt in range(2):
                        dummy = selps.tile([128, ASUB], bf16, tag="scr", name="dummy")
                        nc.vector.tensor_scalar(out=dummy, in0=dA[:, kt, :],
                                                scalar1=tt_[:, kt:kt + 1], scalar2=0.0,
                                                op0=cmp_op, op1=Alu.add, accum_out=cnt[:, kt:kt + 1])
                    u = sel.tile([128, 2], f32, tag="u", name="u")
                    nc.vector.tensor_tensor(out=u, in0=tt_, in1=mu, op=Alu.subtract)
                    nc.vector.tensor_tensor(out=u, in0=u, in1=invsd, op=Alu.mult)
                    nc.vector.tensor_tensor(out=u, in0=u, in1=u, op=Alu.mult)
                    pdf = sel.tile([128, 2], f32, tag="pdf", name="pdf")
                    nc.scalar.activation(pdf, u, Act.Exp, scale=-0.5)
                    nc.vector.tensor_tensor(out=pdf, in0=pdf, in1=invsd, op=Alu.mult)
                    nc.vector.tensor_scalar(out=pdf, in0=pdf, scalar1=0.3989423 * ASUB, scalar2=None,
                                            op0=Alu.mult)
                    ipdf = sel.tile([128, 2], f32, tag="ipdf", name="ipdf")
                    nc.vector.reciprocal_approx_fast(ipdf, pdf)
                    step = sel.tile([128, 2], f32, tag="step", name="step")
                    nc.vector.tensor_scalar(out=step, in0=cnt, scalar1=float(targ), scalar2=None,
                                            op0=Alu.subtract)
                    nc.vector.tensor_tensor(out=step, in0=step, in1=ipdf, op=Alu.mult)
                    if cmp_op == Alu.is_lt:
                        nc.vector.tensor_scalar(out=step, in0=step, scalar1=-1.0, scalar2=None,
                                                op0=Alu.mult)
                    nc.vector.tensor_tensor(out=step, in0=step, in1=sdh, op=Alu.min)
                    nc.vector.tensor_tensor(out=step, in0=step, in1=sdhn, op=Alu.max)
                    nc.vector.tensor_tensor(out=tt_, in0=tt_, in1=step, op=Alu.add)

            # ---- allreduce thresholds ----
            tpack = sel.tile([128, 2, 2], f32, tag="tpack", name="tpack")
            nc.vector.tensor_copy(tpack[:, :, 0], th)
            nc.vector.tensor_copy(tpack[:, :, 1], tl)
            nc.sync.dma_start(
                out=bass.AP(tensor=cc_in, offset=0, ap=[[2, 128], [256, 2], [1, 2]]),
                in_=tpack)
            nc.gpsimd.collective_compute(
                "AllReduce", Alu.add,
                ins=[cc_in.ap()], outs=[cc_out.ap()],
                replica_groups=[list(range(NCORES))])
            tbar = sel.tile([128, 2, 2], f32, tag="tbar", name="tbar")
            nc.sync.dma_start(
                out=tbar,
                in_=bass.AP(tensor=cc_out, offset=0, ap=[[2, 128], [256, 2], [1, 2]]))
            nc.vector.tensor_scalar(out=tbar, in0=tbar, scalar1=1.0 / NCORES, scalar2=None, op0=Alu.mult)
            thb = sel.tile([128, 2], f32, tag="thb", name="thb")
            nc.vector.tensor_copy(thb, tbar[:, :, 0])
            tlb = sel.tile([128, 2], f32, tag="tlb", name="tlb")
            nc.vector.tensor_copy(tlb, tbar[:, :, 1])
            tlbs = sel.tile([128, 2], f32, tag="tlbs", name="tlbs")
            nc.vector.tensor_scalar(out=tlbs, in0=tlb, scalar1=-1.0 / TAU, scalar2=None, op0=Alu.mult)

            # ---- phase B stub ----
            ahi = sel.tile([128, 2], f32, tag="ahi", name="ahi")
            nc.vector.memset(ahi, 0.0)
            alo = sel.tile([128, 2], f32, tag="alo", name="alo")
            nc.vector.memset(alo, 0.0)

            for r, t in [(0, ahi), (1, alo), (2, thb), (3, tlb)]:
                nc.sync.dma_start(
                    out=bass.AP(tensor=cstats_d, offset=r * K, ap=[[1, 128], [128, 2]]),
                    in_=t)
            qtot = sel.tile([E, 1], f32, tag="qtot", name="qtot")
            nc.vector.tensor_reduce(out=qtot, in_=qsums, axis=mybir.AxisListType.X, op=Alu.add)
            rtot = sel.tile([ACT_D, 1], f32, tag="rtot", name="rtot")
            nc.vector.tensor_reduce(out=rtot, in_=rsums, axis=mybir.AxisListType.X, op=Alu.add)
            nc.sync.dma_start(out=bass.AP(tensor=lsums_d, offset=0, ap=[[1, E]]), in_=qtot)
            nc.sync.dma_start(out=bass.AP(tensor=lsums_d, offset=16, ap=[[1, ACT_D]]), in_=rtot)

    nc.compile()
    _CACHED_NC = nc
    return nc


# ---------------------------------------------------------------- host wrapper

def kernel(actions, conditions, enc_w1, enc_b1, enc_w2, enc_b2, enc_w3, enc_b3,
           dec_w1, dec_b1, dec_w2, dec_b2, dec_w3, dec_b3, embedding):
    actions = np.asarray(actions, dtype=np.float32)
    conditions = np.asarray(conditions, dtype=np.float32)
    enc_w1 = np.asarray(enc_w1, dtype=np.float32)
    enc_b1 = np.asarray(enc_b1, dtype=np.float32)
    enc_w2 = np.asarray(enc_w2, dtype=np.float32)
    enc_b2 = np.asarray(enc_b2, dtype=np.float32)
    enc_w3 = np.asarray(enc_w3, dtype=np.float32)
    enc_b3 = np.asarray(enc_b3, dtype=np.float32)
    dec_w1 = np.asarray(dec_w1, dtype=np.float32)
    dec_b1 = np.asarray(dec_b1, dtype=np.float32)
    dec_w2 = np.asarray(dec_w2, dtype=np.float32)
    dec_b2 = np.asarray(dec_b2, dtype=np.float32)
    dec_w3 = np.asarray(dec_w3, dtype=np.float32)
    dec_b3 = np.asarray(dec_b3, dtype=np.float32)
    embedding = np.asarray(embedding, dtype=np.float32)

    # ---- weight prep ----
    Wc = enc_w1[ACT_D:, :]
    Wa = enc_w1[:ACT_D, :]
    Wch, Wcl = _split_f32r(Wc)
    Wah, Wal = _split_f32r(Wa)
    b1h, b1l = _split_f32r(enc_b1[None, :])
    w1s = np.zeros((5 * 128, H1), np.float32)
    w1s[0:128] = Wch[0:128]
    w1s[128:256] = Wch[128:256]
    w1s[256:384] = Wcl[0:128]
    w1s[384:512] = Wcl[128:256]
    w1s[512:524] = Wah
    w1s[524:525] = b1h
    w1s[525:537] = Wal
    w1s[537:538] = b1l
    w1s[544:556] = Wah
    w2h, w2l = _split_f32r(enc_w2)
    w3h, w3l = _split_f32r(enc_w3)
    b2p = (enc_b2.astype(np.float64) - enc_w2.astype(np.float64).sum(0)).astype(np.float32)[:, None]
    b3p = (enc_b3.astype(np.float64) - enc_w3.astype(np.float64).sum(0)).astype(np.float32)[:, None]
    wn = embedding / np.maximum(np.linalg.norm(embedding, axis=1, keepdims=True), 1e-12)
    wTh, wTl = _split_f32r(np.ascontiguousarray(wn.T))
    wstk = np.zeros((WSTK_R, K), np.float32)
    wstk[0:16] = wTh
    wstk[32:48] = wTh
    wstk[64:80] = wTl
    embT = np.ascontiguousarray(embedding.T)
    ones16 = np.ones((E, 1), np.float32)
    DW1 = np.concatenate([dec_w1[E:, :], dec_w1[:E, :]], 0)
    dw1 = _f16_raw(DW1)
    dw2 = _f16_raw(dec_w2)
    dw3 = _f16_raw(dec_w3)
    db2p = (dec_b2.astype(np.float64) - dec_w2.astype(np.float16).astype(np.float64).sum(0)).astype(np.float32)[:, None]
    db3p = (dec_b3.astype(np.float64) - dec_w3.astype(np.float16).astype(np.float64).sum(0)).astype(np.float32)[:, None]

    shared = dict(w1s=w1s, w2h=w2h, w2l=w2l, w3h=w3h, w3l=w3l,
                  b2p=b2p, b3p=b3p, wTh=wTh, wTl=wTl, wstk=wstk, embT=embT, ones16=ones16,
                  dw1=dw1, dw2=dw2, dw3=dw3, db1=dec_b1[:, None].astype(np.float32),
                  db2p=db2p, db3p=db3p)

    in_maps = []
    for i in range(NCORES):
        sl = slice(i * NLOC, (i + 1) * NLOC)
        condT = np.ascontiguousarray(conditions[sl].T)
        chh, cll = _split_f32r(condT)
        actT = np.ascontiguousarray(actions[sl].T)
        ah, al = _split_f32r(actT)
        xmix = np.zeros((XMIX_R, NLOC), np.float32)
        xmix[0:12] = ah
        xmix[12] = 1.0
        xmix[13:25] = ah
        xmix[25] = 1.0
        xmix[32:44] = al
        m = dict(shared)
        m["condh"] = chh
        m["condl"] = cll
        m["xmix"] = xmix
        m["actl"] = np.ascontiguousarray(al)
        in_maps.append(m)

    nc = _build()
    res = run_bass_kernel_spmd(nc, in_maps, core_ids=list(range(NCORES)), trace=TRACE)
    results = res.results
    kernel._last_exec_time_ns = res.exec_time_ns
    kernel._last_results = results

    # ---- host unshard / finish ----
    reconstructed = np.concatenate([r["reconT"].T for r in results], 0)
    quantized_st = np.concatenate([r["qT"].T for r in results], 0)
    idx = np.concatenate([r["idx"] for r in results], 0).astype(np.int32)

    qsum = np.sum([r["lsums"][0:16] for r in results], axis=(0, 1))
    rsum = np.sum([r["lsums"][16:28] for r in results], axis=(0, 1))
    q_latent = np.float32(qsum / (N * E))
    e_latent = np.float32(COMMIT * (qsum / (N * E)))
    recon_loss = np.float32(rsum / (N * ACT_D))

    counts = np.bincount(idx, minlength=K).astype(np.float64)
    avg = counts / N
    perplexity = np.float32(np.exp(-np.sum(avg * np.log(avg + 1e-10))))

    A_hi = np.sum([r["cstats"][0] for r in results], 0).astype(np.float64)
    A_lo = np.sum([r["cstats"][1] for r in results], 0).astype(np.float64)
    t_hi = results[0]["cstats"][2].astype(np.float64)
    t_lo = results[0]["cstats"][3].astype(np.float64)
    n_pos = N // K
    pos = (A_hi - (N - n_pos) * t_hi) / n_pos
    S = A_lo - N / 2.0
    lse = t_lo / TAU + np.log(np.exp((pos - t_lo) / TAU) + S)
    contra = np.float32(np.mean(lse - pos / TAU))

    return (reconstructed, quantized_st, idx, q_latent, e_latent, contra,
            np.float32(perplexity), recon_loss)


kernel._last_exec_time_ns = None
kernel._last_results = None


# revision 26
# speedup vs baseline: 1.2580x; 1.0132x over previous
"""Trainium2 Bass kernel for nn_ActionQuantizer (vq_codebook).

Self-contained: hardcodes shapes/sharding. Accepts FULL inputs, returns FULL outputs.
Data-parallel over batch N across 8 cores; weights/codebook replicated.

v2: K-stacked f32r 3-term matmuls, DRAM-resident selection matrix, deeper buffering.
"""
import sys
sys.path.insert(0, "/opt/trn_rl_repo")

import numpy as np
import concourse.bass as bass
from concourse import bacc
import concourse.mybir as mybir
from concourse.tile import TileContext
from concourse.bass_utils import run_bass_kernel_spmd

# ---- problem constants ----
N = 131072
ACT_D = 12
COND_D = 256
K = 256            # codebook size
E = 16             # embedding dim
H1, H2 = 512, 256
COMMIT = 0.25
TAU = 0.07

NCORES = 8
NLOC = N // NCORES          # 16384
CH = 512                    # batch chunk (moving dim)
NCHUNK = NLOC // CH         # 32
ASUB = 4096                 # phase-A subsample columns
NEWTON_ITERS = 4
XMIX_R = 44                 # packed [act_h;1;act_h;1;pad;act_l]
WSTK_R = 80                 # packed [wTh;0;wTh;0;wTl] for batch-major dist

f32 = mybir.dt.float32
f32r = mybir.dt.float32r
bf16 = mybir.dt.bfloat16
f16 = mybir.dt.float16
i32 = mybir.dt.int32
i16 = mybir.dt.int16
u32 = mybir.dt.uint32
Alu = mybir.AluOpType
Act = mybir.ActivationFunctionType

DEBUG = False
TRACE = False
SELECT = True

# ---------------------------------------------------------------- host helpers

def _split_f32r(x):
    """Lossless split x = hi + lo with both f32r-legal (low 12 mantissa bits zero)."""
    x = np.ascontiguousarray(x, dtype=np.float32)
    b = x.view(np.uint32)
    hi = (b & np.uint32(0xFFFFF000)).view(np.float32)
    lo = (x - hi).astype(np.float32)
    return hi, lo


def _f16_raw(x):
    return np.ascontiguousarray(np.asarray(x, dtype=np.float32).astype(np.float16))


# ---------------------------------------------------------------- kernel build

_CACHED_NC = None

def _build():
    global _CACHED_NC
    if _CACHED_NC is not None:
        return _CACHED_NC
    nc = bacc.Bacc("TRN2", target_bir_lowering=False, num_devices=NCORES)

    # ---- dram inputs ----
    ch_d = nc.dram_tensor("condh", [COND_D, NLOC], f32r, kind="ExternalInput")
    cl_d = nc.dram_tensor("condl", [COND_D, NLOC], f32r, kind="ExternalInput")
    xmix_d = nc.dram_tensor("xmix", [XMIX_R, NLOC], f32r, kind="ExternalInput")
    actl_d = nc.dram_tensor("actl", [ACT_D, NLOC], f32, kind="ExternalInput")
    w1s_d = nc.dram_tensor("w1s", [5 * 128, H1], f32r, kind="ExternalInput")
    w2h_d = nc.dram_tensor("w2h", [H1, H2], f32r, kind="ExternalInput")
    w2l_d = nc.dram_tensor("w2l", [H1, H2], f32r, kind="ExternalInput")
    w3h_d = nc.dram_tensor("w3h", [H2, E], f32r, kind="ExternalInput")
    w3l_d = nc.dram_tensor("w3l", [H2, E], f32r, kind="ExternalInput")
    b2p_d = nc.dram_tensor("b2p", [H2, 1], f32, kind="ExternalInput")
    b3p_d = nc.dram_tensor("b3p", [E, 1], f32, kind="ExternalInput")
    wTh_d = nc.dram_tensor("wTh", [E, K], f32r, kind="ExternalInput")
    wTl_d = nc.dram_tensor("wTl", [E, K], f32r, kind="ExternalInput")
    wstk_d = nc.dram_tensor("wstk", [WSTK_R, K], f32r, kind="ExternalInput")
    embT_d = nc.dram_tensor("embT", [E, K], f32, kind="ExternalInput")
    ones16_d = nc.dram_tensor("ones16", [E, 1], f32r, kind="ExternalInput")
    dw1_d = nc.dram_tensor("dw1", [COND_D + E, H2], f16, kind="ExternalInput")  # [cond; q]
    db1_d = nc.dram_tensor("db1", [H2, 1], f32, kind="ExternalInput")
    dw2_d = nc.dram_tensor("dw2", [H2, H1], f16, kind="ExternalInput")
    dw3_d = nc.dram_tensor("dw3", [H1, ACT_D], f16, kind="ExternalInput")
    db2p_d = nc.dram_tensor("db2p", [H1, 1], f32, kind="ExternalInput")
    db3p_d = nc.dram_tensor("db3p", [ACT_D, 1], f32, kind="ExternalInput")

    # ---- dram outputs ----
    recon_d = nc.dram_tensor("reconT", [ACT_D, NLOC], f32, kind="ExternalOutput")
    qT_d = nc.dram_tensor("qT", [E, NLOC], f32, kind="ExternalOutput")
    idx_d = nc.dram_tensor("idx", [NLOC], i32, kind="ExternalOutput")
    cstats_d = nc.dram_tensor("cstats", [4, K], f32, kind="ExternalOutput")
    lsums_d = nc.dram_tensor("lsums", [32], f32, kind="ExternalOutput")
    if DEBUG:
        zdbg_d = nc.dram_tensor("zdbg", [E, NLOC], f32, kind="ExternalOutput")

    # ---- internal dram ----
    dcm_d = nc.dram_tensor("dcm", [128, 2, NLOC], bf16)
    invn_d = nc.dram_tensor("invnscr", [NCHUNK, CH], f32)
    cc_in = nc.dram_tensor("cc_in", [K, 2], f32)
    cc_out = nc.dram_tensor("cc_out", [K, 2], f32, addr_space="Shared")

    with TileContext(nc) as tc:
        import contextlib
        ctx = contextlib.ExitStack()
        with ctx:
            singles = ctx.enter_context(tc.tile_pool(name="singles", bufs=1))
            loopctx = ctx.enter_context(contextlib.ExitStack())
            chunkio = loopctx.enter_context(tc.tile_pool(name="chunkio", bufs=2))
            work = loopctx.enter_context(tc.tile_pool(name="work", bufs=1))
            dbl = loopctx.enter_context(tc.tile_pool(name="dbl", bufs=2))
            small = loopctx.enter_context(tc.tile_pool(name="small", bufs=2))
            pm = loopctx.enter_context(tc.tile_pool(name="pm", bufs=3, space="PSUM"))
            psml = loopctx.enter_context(tc.tile_pool(name="psml", bufs=2, space="PSUM"))

            def load(pool, dram, shape, tag):
                t = pool.tile(shape, dram.dtype, tag=tag, name=tag)
                nc.sync.dma_start(out=t, in_=dram.ap())
                return t

            def load_kt(dram, ktiles, free, dtype, tag, rows=None):
                t = singles.tile([128, ktiles, free], dtype, tag=tag, name=tag)
                nrows = rows or dram.shape[0]
                for k_ in range(ktiles):
                    r0 = 128 * k_
                    r1 = min(r0 + 128, nrows)
                    if r1 > r0:
                        nc.sync.dma_start(out=t[0:r1 - r0, k_, :], in_=dram[r0:r1, :])
                return t

            def load_scal(dram, T, tag):
                t = singles.tile([128, T, 1], f32, tag=tag, name=tag)
                nc.sync.dma_start(out=t, in_=bass.AP(tensor=dram, offset=0,
                                                     ap=[[1, 128], [128, T], [1, 1]]))
                return t

            w1s = load_kt(w1s_d, 5, H1, f32r, "w1s")
            w2h = load_kt(w2h_d, 4, H2, f32r, "w2h")
            w2l = load_kt(w2l_d, 4, H2, f32r, "w2l")
            w3h = load_kt(w3h_d, 2, E, f32r, "w3h")
            w3l = load_kt(w3l_d, 2, E, f32r, "w3l")
            b2p = load_scal(b2p_d, 2, "b2p")
            b2p1 = singles.tile([128, 2, 1], f32, tag="b2p1", name="b2p1")
            nc.vector.tensor_scalar(out=b2p1, in0=b2p, scalar1=1.0, scalar2=None, op0=Alu.add)
            b3p = load(singles, b3p_d, [E, 1], "b3p")
            wTh = load(singles, wTh_d, [E, K], "wTh")
            wTl = load(singles, wTl_d, [E, K], "wTl")
            wstk = load(singles, wstk_d, [WSTK_R, K], "wstk")
            embT = load(singles, embT_d, [E, K], "embT")
            ones16 = load(singles, ones16_d, [E, 1], "ones16")
            dw1 = load_kt(dw1_d, 3, H2, f16, "dw1", rows=COND_D + E)
            dw2 = load_kt(dw2_d, 2, H1, f16, "dw2")
            dw3 = load_kt(dw3_d, 4, ACT_D, f16, "dw3")
            db1 = load_scal(db1_d, 2, "db1")
            db1p1 = singles.tile([128, 2, 1], f32, tag="db1p1", name="db1p1")
            nc.vector.tensor_scalar(out=db1p1, in0=db1, scalar1=1.0, scalar2=None, op0=Alu.add)
            db2p = load_scal(db2p_d, 4, "db2p")
            db2p1 = singles.tile([128, 4, 1], f32, tag="db2p1", name="db2p1")
            nc.vector.tensor_scalar(out=db2p1, in0=db2p, scalar1=1.0, scalar2=None, op0=Alu.add)
            db3p = load(singles, db3p_d, [ACT_D, 1], "db3p")

            # z-stacks for batch-major distances (double-buffered by parity)
            zstks = []
            for pz in range(2):
                zs = singles.tile([WSTK_R, CH], f32r, tag=f"zstk{pz}", name=f"zstk{pz}")
                nc.vector.memset(zs.bitcast(f32), 0.0)
                zstks.append(zs)

            qsums = singles.tile([E, NCHUNK], f32, tag="qsums", name="qsums")
            rsums = singles.tile([ACT_D, NCHUNK], f32, tag="rsums", name="rsums")

            # ================= main chunk loop =================
            for c in range(NCHUNK):
                cs0 = c * CH
                ch0 = chunkio.tile([128, CH], f32r, tag="ch0", name="ch0")
                nc.sync.dma_start(out=ch0, in_=ch_d[0:128, cs0:cs0 + CH])
                ch1 = chunkio.tile([128, CH], f32r, tag="ch1", name="ch1")
                nc.sync.dma_start(out=ch1, in_=ch_d[128:256, cs0:cs0 + CH])
                cl0 = chunkio.tile([128, CH], f32r, tag="cl0", name="cl0")
                nc.sync.dma_start(out=cl0, in_=cl_d[0:128, cs0:cs0 + CH])
                cl1 = chunkio.tile([128, CH], f32r, tag="cl1", name="cl1")
                nc.sync.dma_start(out=cl1, in_=cl_d[128:256, cs0:cs0 + CH])
                xmx = chunkio.tile([XMIX_R, CH], f32r, tag="xmx", name="xmx")
                nc.sync.dma_start(out=xmx, in_=xmix_d[:, cs0:cs0 + CH])
                actl = chunkio.tile([ACT_D, CH], f32, tag="actl", name="actl")
                nc.sync.dma_start(out=actl, in_=actl_d[:, cs0:cs0 + CH])

                # ---------- enc1 (K-stacked 3-term) ----------
                ps1 = [pm.tile([128, 2, CH], f32, tag="pm", name="ps1a"),
                       pm.tile([128, 2, CH], f32, tag="pm", name="ps1b")]
                for m in range(4):
                    pv = ps1[m // 2][:, m % 2, :]
                    ms = slice(128 * m, 128 * (m + 1))
                    terms = [(w1s[:, 0, ms], ch0), (w1s[:, 1, ms], ch1),
                             (w1s[:, 2, ms], ch0), (w1s[:, 3, ms], ch1),
                             (w1s[:, 0, ms], cl0), (w1s[:, 1, ms], cl1),
                             (w1s[0:XMIX_R, 4, ms], xmx)]
                    for i, (lw, rx) in enumerate(terms):
                        nc.tensor.matmul(pv, lw, rx, start=(i == 0), stop=(i == len(terms) - 1))

                # ---------- ELU1: p = min(exp(s), max(s+1,1)) ----------
                e1 = work.tile([128, 4, CH], f32, tag="e1", name="e1")
                r1 = work.tile([128, 4, CH], f32, tag="r1", name="r1")
                hh1 = dbl.tile([128, 4, CH], f32r, tag="hh1", name="hh1")
                hl1 = dbl.tile([128, 4, CH], f32r, tag="hl1", name="hl1")
                for half in range(2):
                    hs = slice(2 * half, 2 * half + 2)
                    nc.scalar.activation(e1[:, hs, :], ps1[half][:, :, :], Act.Exp)
                    nc.vector.tensor_scalar(out=r1[:, hs, :], in0=ps1[half][:, :, :],
                                            scalar1=1.0, scalar2=1.0, op0=Alu.add, op1=Alu.max)
                nc.vector.tensor_tensor(out=r1, in0=e1, in1=r1, op=Alu.min)
                nc.vector.tensor_copy(hh1, r1)
                nc.vector.tensor_tensor(out=hl1, in0=r1, in1=hh1.bitcast(f32), op=Alu.subtract)

                # ---------- enc2 ----------
                ps2 = pm.tile([128, 2, CH], f32, tag="pm", name="ps2")
                for m in range(2):
                    pv = ps2[:, m, :]
                    ms = slice(128 * m, 128 * (m + 1))
                    terms = []
                    for t in range(4):
                        terms.append((w2h[:, t, ms], hh1[:, t, :]))
                    for t in range(4):
                        terms.append((w2l[:, t, ms], hh1[:, t, :]))
                    for t in range(4):
                        terms.append((w2h[:, t, ms], hl1[:, t, :]))
                    for i, (lw, rx) in enumerate(terms):
                        nc.tensor.matmul(pv, lw, rx, start=(i == 0), stop=(i == len(terms) - 1))

                # ---------- ELU2 ----------
                e2 = work.tile([128, 2, CH], f32, tag="e2", name="e2")
                r2 = work.tile([128, 2, CH], f32, tag="r2", name="r2")
                hh2 = dbl.tile([128, 2, CH], f32r, tag="hh2", name="hh2")
                hl2 = dbl.tile([128, 2, CH], f32r, tag="hl2", name="hl2")
                for m in range(2):
                    nc.scalar.activation(e2[:, m, :], ps2[:, m, :], Act.Exp, bias=b2p[:, m, :])
                    nc.vector.tensor_scalar(out=r2[:, m, :], in0=ps2[:, m, :],
                                            scalar1=b2p1[:, m, :], scalar2=1.0, op0=Alu.add, op1=Alu.max)
                nc.vector.tensor_tensor(out=r2, in0=e2, in1=r2, op=Alu.min)
                nc.vector.tensor_copy(hh2, r2)
                nc.vector.tensor_tensor(out=hl2, in0=r2, in1=hh2.bitcast(f32), op=Alu.subtract)

                # ---------- enc3: z ----------
                psz = psml.tile([E, CH], f32, tag="ps_s", name="psz")
                terms = []
                for t in range(2):
                    terms.append((w3h[:, t, :], hh2[:, t, :]))
                for t in range(2):
                    terms.append((w3l[:, t, :], hh2[:, t, :]))
                for t in range(2):
                    terms.append((w3h[:, t, :], hl2[:, t, :]))
                for i, (lw, rx) in enumerate(terms):
                    nc.tensor.matmul(psz, lw, rx, start=(i == 0), stop=(i == len(terms) - 1))
                z = small.tile([E, CH], f32, tag="z", name="z")
                nc.vector.tensor_scalar(out=z, in0=psz, scalar1=b3p[:, 0:1], scalar2=None, op0=Alu.add)
                if DEBUG:
                    nc.sync.dma_start(out=zdbg_d[:, cs0:cs0 + CH], in_=z)

                # ---------- z stack (raw z hi/lo) for batch-major dist ----------
                zrh = small.tile([E, CH], f32r, tag="zrh", name="zrh")
                nc.vector.tensor_copy(zrh, z)
                zrl = small.tile([E, CH], f32r, tag="zrl", name="zrl")
                nc.vector.tensor_tensor(out=zrl, in0=z, in1=zrh.bitcast(f32), op=Alu.subtract)
                # ---------- batch-major distances + argmax ----------
                psbm = pm.tile([128, 4, 256], f32, tag="pm", name="psbm")
                for j in range(4):
                    js = slice(128 * j, 128 * (j + 1))
                    terms = [(zrh[:, js], wTh), (zrl[:, js], wTh), (zrh[:, js], wTl)]
                    for i, (lw, rx) in enumerate(terms):
                        nc.tensor.matmul(psbm[:, j, :], lw, rx, start=(i == 0), stop=(i == 2))
                m8 = small.tile([128, 4, 8], f32, tag="m8", name="m8")
                i8 = small.tile([128, 4, 8], u32, tag="i8", name="i8")
                for j in range(4):
                    nc.vector.max(m8[:, j, :], psbm[:, j, :])
                    nc.vector.max_index(i8[:, j, :], m8[:, j, :], psbm[:, j, :])
                idx32 = small.tile([128, 4], i32, tag="idx32", name="idx32")
                nc.vector.tensor_copy(idx32, i8[:, :, 0])
                nc.sync.dma_start(
                    out=bass.AP(tensor=idx_d, offset=cs0, ap=[[1, 128], [128, 4]]),
                    in_=idx32)
                idxw32 = small.tile([16, CH // 16], i32, tag="idxw32", name="idxw32")
                nc.sync.dma_start(
                    out=idxw32,
                    in_=bass.AP(tensor=idx_d, offset=cs0, ap=[[1, 16], [16, CH // 16]]))
                idxw = small.tile([16, CH // 16], i16, tag="idxw", name="idxw")
                nc.vector.tensor_copy(idxw, idxw32)

                # ---------- normalize (selection path only) ----------
                zzr = small.tile([E, CH], f32r, tag="zzr", name="zzr")
                nc.vector.tensor_tensor(out=zzr, in0=z, in1=z, op=Alu.mult)
                psn = psml.tile([1, CH], f32, tag="ps_s", name="psn")
                nc.tensor.matmul(psn, ones16, zzr, start=True, stop=True)
                nrm = small.tile([1, CH], f32, tag="nrm", name="nrm")
                nc.scalar.activation(nrm, psn, Act.Sqrt)
                invn1 = small.tile([1, CH], f32, tag="invn1", name="invn1")
                nc.vector.reciprocal_approx_fast(invn1, nrm)
                invn16 = small.tile([E, CH], f32, tag="invn16", name="invn16")
                nc.gpsimd.partition_broadcast(invn16, invn1, channels=E)
                zhat = small.tile([E, CH], f32, tag="zhat", name="zhat")
                nc.vector.tensor_tensor(out=zhat, in0=z, in1=invn16, op=Alu.mult)
                zh = small.tile([E, CH], f32r, tag="zh", name="zh")
                nc.vector.tensor_copy(zh, zhat)

                # ---------- distances code-major (selection, 1-term) ----------
                pscm = pm.tile([128, 2, CH], f32, tag="pm", name="pscm")
                for mcode in range(2):
                    nc.tensor.matmul(pscm[:, mcode, :], wTh[:, 128 * mcode:128 * (mcode + 1)], zh,
                                     start=True, stop=True)
                stg = small.tile([128, 2, CH], bf16, tag="stg", name="stg")
                nc.scalar.copy(stg, pscm)
                nc.sync.dma_start(out=dcm_d[:, :, cs0:cs0 + CH], in_=stg)

                # ---------- gather quantized ----------
                gsb = small.tile([E, CH], f32, tag="zhat", name="gsb")
                nc.gpsimd.ap_gather(gsb, embT, idxw, channels=16, num_elems=K, d=1, num_idxs=CH)
                nc.sync.dma_start(out=qT_d[:, cs0:cs0 + CH], in_=gsb)
                qd = small.tile([E, CH], f32, tag="zzr", name="qd")
                nc.vector.tensor_tensor(out=qd, in0=gsb, in1=z, op=Alu.subtract)
                nc.scalar.activation(qd, qd, Act.Square, accum_out=qsums[:, c:c + 1])

                # ---------- decoder rhs ----------
                cb0 = work.tile([128, CH], f16, tag="cb0", name="cb0")
                nc.vector.tensor_copy(cb0, ch0.bitcast(f32))
                cb1 = work.tile([128, CH], f16, tag="cb1", name="cb1")
                nc.vector.tensor_copy(cb1, ch1.bitcast(f32))
                dr2 = work.tile([16, CH], f16, tag="dr2", name="dr2")
                nc.vector.tensor_copy(dr2, gsb)

                # ---------- dec1 ----------
                psd1 = pm.tile([128, 2, CH], f32, tag="pm", name="psd1")
                for m in range(2):
                    ms = slice(128 * m, 128 * (m + 1))
                    terms = [(dw1[:, 0, ms], cb0), (dw1[:, 1, ms], cb1), (dw1[0:16, 2, ms], dr2)]
                    for i, (lw, rx) in enumerate(terms):
                        nc.tensor.matmul(psd1[:, m, :], lw, rx, start=(i == 0), stop=(i == 2))
                ed1 = work.tile([128, 2, CH], f16, tag="ed1", name="ed1")
                rd1 = work.tile([128, 2, CH], f16, tag="rd1", name="rd1")
                for m in range(2):
                    nc.scalar.activation(ed1[:, m, :], psd1[:, m, :], Act.Exp, bias=db1[:, m, :])
                    nc.vector.tensor_scalar(out=rd1[:, m, :], in0=psd1[:, m, :],
                                            scalar1=db1p1[:, m, :], scalar2=1.0,
                                            op0=Alu.add, op1=Alu.max)
                nc.vector.tensor_tensor(out=rd1, in0=ed1, in1=rd1, op=Alu.min)

                # ---------- dec2 ----------
                psd2 = [pm.tile([128, 2, CH], f32, tag="pm", name="psd2a"),
                        pm.tile([128, 2, CH], f32, tag="pm", name="psd2b")]
                for m in range(4):
                    pv = psd2[m // 2][:, m % 2, :]
                    ms = slice(128 * m, 128 * (m + 1))
                    for t in range(2):
                        nc.tensor.matmul(pv, dw2[:, t, ms], rd1[:, t, :], start=(t == 0), stop=(t == 1))
                ed2 = work.tile([128, 4, CH], f16, tag="ed2", name="ed2")
                rd2 = work.tile([128, 4, CH], f16, tag="rd2", name="rd2")
                for m in range(4):
                    nc.scalar.activation(ed2[:, m, :], psd2[m // 2][:, m % 2, :], Act.Exp,
                                         bias=db2p[:, m, :])
                    nc.vector.tensor_scalar(out=rd2[:, m, :], in0=psd2[m // 2][:, m % 2, :],
                                            scalar1=db2p1[:, m, :], scalar2=1.0,
                                            op0=Alu.add, op1=Alu.max)
                nc.vector.tensor_tensor(out=rd2, in0=ed2, in1=rd2, op=Alu.min)

                # ---------- dec3 ----------
                psd3 = psml.tile([ACT_D, CH], f32, tag="ps_s", name="psd3")
                for t in range(4):
                    nc.tensor.matmul(psd3, dw3[:, t, 0:ACT_D], rd2[:, t, :], start=(t == 0), stop=(t == 3))
                recon = small.tile([ACT_D, CH], f32, tag="recon", name="recon")
                nc.vector.tensor_scalar(out=recon, in0=psd3, scalar1=db3p[:, 0:1], scalar2=None, op0=Alu.add)
                nc.sync.dma_start(out=recon_d[:, cs0:cs0 + CH], in_=recon)
                rdf = small.tile([ACT_D, CH], f32, tag="z", name="rdf")
                nc.vector.tensor_tensor(out=rdf, in0=recon, in1=xmx[0:12, :].bitcast(f32), op=Alu.subtract)
                nc.vector.tensor_tensor(out=rdf, in0=rdf, in1=actl, op=Alu.subtract)
                nc.scalar.activation(rdf, rdf, Act.Square, accum_out=rsums[:, c:c + 1])

            # ================= selection =================
            loopctx.close()
            if not SELECT:
                _CACHED = None
            sel = ctx.enter_context(tc.tile_pool(name="sel", bufs=1))
            selps = ctx.enter_context(tc.tile_pool(name="selps", bufs=2))

            # ---- phase A on the first ASUB columns (from DRAM) ----
            dA = sel.tile([128, 2, ASUB], bf16, tag="dA", name="dA")
            nc.sync.dma_start(out=dA, in_=dcm_d[:, :, 0:ASUB])

            s1 = sel.tile([128, 2], f32, tag="s1", name="s1")
            s2 = sel.tile([128, 2], f32, tag="s2", name="s2")
            s2c = sel.tile([128, 2, 2], f32, tag="s2c", name="s2c")
            for kt in range(2):
                dummy = selps.tile([128, ASUB], bf16, tag="scr", name="dummy")
                nc.vector.tensor_scalar(out=dummy, in0=dA[:, kt, :], scalar1=0.0, scalar2=0.0,
                                        op0=Alu.add, op1=Alu.add, accum_out=s1[:, kt:kt + 1])
                for h in range(2):
                    sq = selps.tile([128, ASUB // 2], f32, tag="scr", name="sq")
                    hs = slice(h * (ASUB // 2), (h + 1) * (ASUB // 2))
                    nc.scalar.activation(sq, dA[:, kt, hs], Act.Square,
                                         accum_out=s2c[:, kt, h:h + 1])
            nc.vector.tensor_reduce(out=s2, in_=s2c, axis=mybir.AxisListType.X, op=Alu.add)
            mu = sel.tile([128, 2], f32, tag="mu", name="mu")
            nc.vector.tensor_scalar(out=mu, in0=s1, scalar1=1.0 / ASUB, scalar2=None, op0=Alu.mult)
            var = sel.tile([128, 2], f32, tag="var", name="var")
            nc.vector.tensor_scalar(out=var, in0=s2, scalar1=1.0 / ASUB, scalar2=None, op0=Alu.mult)
            mumu = sel.tile([128, 2], f32, tag="mumu", name="mumu")
            nc.vector.tensor_tensor(out=mumu, in0=mu, in1=mu, op=Alu.mult)
            nc.vector.tensor_tensor(out=var, in0=var, in1=mumu, op=Alu.subtract)
            sd = sel.tile([128, 2], f32, tag="sd", name="sd")
            nc.scalar.activation(sd, var, Act.Sqrt)
            invsd = sel.tile([128, 2], f32, tag="invsd", name="invsd")
            nc.vector.reciprocal_approx_fast(invsd, sd)
            sdh = sel.tile([128, 2], f32, tag="sdh", name="sdh")
            nc.vector.tensor_scalar(out=sdh, in0=sd, scalar1=0.5, scalar2=None, op0=Alu.mult)
            sdhn = sel.tile([128, 2], f32, tag="sdhn", name="sdhn")
            nc.vector.tensor_scalar(out=sdhn, in0=sdh, scalar1=-1.0, scalar2=None, op0=Alu.mult)

            th = sel.tile([128, 2], f32, tag="th", name="th")
            tsc = sel.tile([128, 2], f32, tag="tsc", name="tsc")
            nc.vector.tensor_scalar(out=tsc, in0=sd, scalar1=2.653, scalar2=None, op0=Alu.mult)
            nc.vector.tensor_tensor(out=th, in0=mu, in1=tsc, op=Alu.add)
            tl = sel.tile([128, 2], f32, tag="tl", name="tl")
            nc.vector.tensor_copy(tl, mu)

            cnt = sel.tile([128, 2], f32, tag="cnt", name="cnt")
            targ_hi = ASUB * 512.0 / N
            targ_lo = ASUB * 0.5
            for it in range(NEWTON_ITERS):
                for (tt_, cmp_op, targ) in [(th, Alu.is_gt, targ_hi), (tl, Alu.is_lt, targ_lo)]:
                    for kt in range(2):
                        dummy = selps.tile([128, ASUB], bf16, tag="scr", name="dummy")
                        nc.vector.tensor_scalar(out=dummy, in0=dA[:, kt, :],
                                                scalar1=tt_[:, kt:kt + 1], scalar2=0.0,
                                                op0=cmp_op, op1=Alu.add, accum_out=cnt[:, kt:kt + 1])
                    u = sel.tile([128, 2], f32, tag="u", name="u")
                    nc.vector.tensor_tensor(out=u, in0=tt_, in1=mu, op=Alu.subtract)
                    nc.vector.tensor_tensor(out=u, in0=u, in1=invsd, op=Alu.mult)
                    nc.vector.tensor_tensor(out=u, in0=u, in1=u, op=Alu.mult)
                    pdf = sel.tile([128, 2], f32, tag="pdf", name="pdf")
                    nc.scalar.activation(pdf, u, Act.Exp, scale=-0.5)
                    nc.vector.tensor_tensor(out=pdf, in0=pdf, in1=invsd, op=Alu.mult)
                    nc.vector.tensor_scalar(out=pdf, in0=pdf, scalar1=0.3989423 * ASUB, scalar2=None,
                                            op0=Alu.mult)
                    ipdf = sel.tile([128, 2], f32, tag="ipdf", name="ipdf")
                    nc.vector.reciprocal_approx_fast(ipdf, pdf)
                    step = sel.tile([128, 2], f32, tag="step", name="step")
                    nc.vector.tensor_scalar(out=step, in0=cnt, scalar1=float(targ), scalar2=None,
                                            op0=Alu.subtract)
                    nc.vector.tensor_tensor(out=step, in0=step, in1=ipdf, op=Alu.mult)
                    if cmp_op == Alu.is_lt:
                        nc.vector.tensor_scalar(out=step, in0=step, scalar1=-1.0, scalar2=None,
                                                op0=Alu.mult)
                    nc.vector.tensor_tensor(out=step, in0=step, in1=sdh, op=Alu.min)
                    nc.vector.tensor_tensor(out=step, in0=step, in1=sdhn, op=Alu.max)
                    nc.vector.tensor_tensor(out=tt_, in0=tt_, in1=step, op=Alu.add)

            # ---- allreduce thresholds ----
            tpack = sel.tile([128, 2, 2], f32, tag="tpack", name="tpack")
            nc.vector.tensor_copy(tpack[:, :, 0], th)
            nc.vector.tensor_copy(tpack[:, :, 1], tl)
            nc.sync.dma_start(
                out=bass.AP(tensor=cc_in, offset=0, ap=[[2, 128], [256, 2], [1, 2]]),
                in_=tpack)
            nc.gpsimd.collective_compute(
                "AllReduce", Alu.add,
                ins=[cc_in.ap()], outs=[cc_out.ap()],
                replica_groups=[list(range(NCORES))])
            tbar = sel.tile([128, 2, 2], f32, tag="tbar", name="tbar")
            nc.sync.dma_start(
                out=tbar,
                in_=bass.AP(tensor=cc_out, offset=0, ap=[[2, 128], [256, 2], [1, 2]]))
            nc.vector.tensor_scalar(out=tbar, in0=tbar, scalar1=1.0 / NCORES, scalar2=None, op0=Alu.mult)
            thb = sel.tile([128, 2], f32, tag="thb", name="thb")
            nc.vector.tensor_copy(thb, tbar[:, :, 0])
            tlb = sel.tile([128, 2], f32, tag="tlb", name="tlb")
            nc.vector.tensor_copy(tlb, tbar[:, :, 1])
            tlbs = sel.tile([128, 2], f32, tag="tlbs", name="tlbs")
            nc.vector.tensor_scalar(out=tlbs, in0=tlb, scalar1=-1.0 / TAU, scalar2=None, op0=Alu.mult)

            # ---- phase B: masked sums over full data (stream from DRAM) ----
            BCH = 2048
            nbc = NLOC // BCH
            ahic = sel.tile([128, 2, nbc], f32, tag="ahic", name="ahic")
            aloc = sel.tile([128, 2, nbc], f32, tag="aloc", name="aloc")
            for g in range(nbc):
                g0 = g * BCH
                dg = selps.tile([128, 2, BCH], bf16, tag="dg", name="dg")
                nc.sync.dma_start(out=dg, in_=dcm_d[:, :, g0:g0 + BCH])
                exg = selps.tile([128, 2, BCH], bf16, tag="scr", name="exg")
                for kt in range(2):
                    nc.scalar.activation(exg[:, kt, :], dg[:, kt, :], Act.Exp,
                                         bias=tlbs[:, kt:kt + 1], scale=1.0 / TAU)
                    dummy = selps.tile([128, BCH], bf16, tag="scr2", name="dummy")
                    nc.vector.tensor_scalar(out=dummy, in0=dg[:, kt, :],
                                            scalar1=thb[:, kt:kt + 1], scalar2=0.0,
                                            op0=Alu.max, op1=Alu.add,
                                            accum_out=ahic[:, kt, g:g + 1])
                    dummy2 = selps.tile([128, BCH], bf16, tag="scr2", name="dummy2")
                    nc.vector.tensor_scalar(out=dummy2, in0=exg[:, kt, :],
                                            scalar1=1.0, scalar2=0.0,
                                            op0=Alu.min, op1=Alu.add,
                                            accum_out=aloc[:, kt, g:g + 1])
            ahi = sel.tile([128, 2], f32, tag="ahi", name="ahi")
            nc.vector.tensor_reduce(out=ahi, in_=ahic, axis=mybir.AxisListType.X, op=Alu.add)
            alo = sel.tile([128, 2], f32, tag="alo", name="alo")
            nc.vector.tensor_reduce(out=alo, in_=aloc, axis=mybir.AxisListType.X, op=Alu.add)

            for r, t in [(0, ahi), (1, alo), (2, thb), (3, tlb)]:
                nc.sync.dma_start(
                    out=bass.AP(tensor=cstats_d, offset=r * K, ap=[[1, 128], [128, 2]]),
                    in_=t)
            qtot = sel.tile([E, 1], f32, tag="qtot", name="qtot")
            nc.vector.tensor_reduce(out=qtot, in_=qsums, axis=mybir.AxisListType.X, op=Alu.add)
            rtot = sel.tile([ACT_D, 1], f32, tag="rtot", name="rtot")
            nc.vector.tensor_reduce(out=rtot, in_=rsums, axis=mybir.AxisListType.X, op=Alu.add)
            nc.sync.dma_start(out=bass.AP(tensor=lsums_d, offset=0, ap=[[1, E]]), in_=qtot)
            nc.sync.dma_start(out=bass.AP(tensor=lsums_d, offset=16, ap=[[1, ACT_D]]), in_=rtot)

    nc.compile()
    _CACHED_NC = nc
    return nc


# ---------------------------------------------------------------- host wrapper

def kernel(actions, conditions, enc_w1, enc_b1, enc_w2, enc_b2, enc_w3, enc_b3,
           dec_w1, dec_b1, dec_w2, dec_b2, dec_w3, dec_b3, embedding):
    actions = np.asarray(actions, dtype=np.float32)
    conditions = np.asarray(conditions, dtype=np.float32)
    enc_w1 = np.asarray(enc_w1, dtype=np.float32)
    enc_b1 = np.asarray(enc_b1, dtype=np.float32)
    enc_w2 = np.asarray(enc_w2, dtype=np.float32)
    enc_b2 = np.asarray(enc_b2, dtype=np.float32)
    enc_w3 = np.asarray(enc_w3, dtype=np.float32)
    enc_b3 = np.asarray(enc_b3, dtype=np.float32)
    dec_w1 = np.asarray(dec_w1, dtype=np.float32)
    dec_b1 = np.asarray(dec_b1, dtype=np.float32)
    dec_w2 = np.asarray(dec_w2, dtype=np.float32)
    dec_b2 = np.asarray(dec_b2, dtype=np.float32)
    dec_w3 = np.asarray(dec_w3, dtype=np.float32)
    dec_b3 = np.asarray(dec_b3, dtype=np.float32)
    embedding = np.asarray(embedding, dtype=np.float32)

    # ---- weight prep ----
    Wc = enc_w1[ACT_D:, :]
    Wa = enc_w1[:ACT_D, :]
    Wch, Wcl = _split_f32r(Wc)
    Wah, Wal = _split_f32r(Wa)
    b1h, b1l = _split_f32r(enc_b1[None, :])
    w1s = np.zeros((5 * 128, H1), np.float32)
    w1s[0:128] = Wch[0:128]
    w1s[128:256] = Wch[128:256]
    w1s[256:384] = Wcl[0:128]
    w1s[384:512] = Wcl[128:256]
    w1s[512:524] = Wah
    w1s[524:525] = b1h
    w1s[525:537] = Wal
    w1s[537:538] = b1l
    w1s[544:556] = Wah
    w2h, w2l = _split_f32r(enc_w2)
    w3h, w3l = _split_f32r(enc_w3)
    b2p = (enc_b2.astype(np.float64) - enc_w2.astype(np.float64).sum(0)).astype(np.float32)[:, None]
    b3p = (enc_b3.astype(np.float64) - enc_w3.astype(np.float64).sum(0)).astype(np.float32)[:, None]
    wn = embedding / np.maximum(np.linalg.norm(embedding, axis=1, keepdims=True), 1e-12)
    wTh, wTl = _split_f32r(np.ascontiguousarray(wn.T))
    wstk = np.zeros((WSTK_R, K), np.float32)
    wstk[0:16] = wTh
    wstk[32:48] = wTh
    wstk[64:80] = wTl
    embT = np.ascontiguousarray(embedding.T)
    ones16 = np.ones((E, 1), np.float32)
    DW1 = np.concatenate([dec_w1[E:, :], dec_w1[:E, :]], 0)
    dw1 = _f16_raw(DW1)
    dw2 = _f16_raw(dec_w2)
    dw3 = _f16_raw(dec_w3)
    db2p = (dec_b2.astype(np.float64) - dec_w2.astype(np.float16).astype(np.float64).sum(0)).astype(np.float32)[:, None]
    db3p = (dec_b3.astype(np.float64) - dec_w3.astype(np.float16).astype(np.float64).sum(0)).astype(np.float32)[:, None]

    shared = dict(w1s=w1s, w2h=w2h, w2l=w2l, w3h=w3h, w3l=w3l,
                  b2p=b2p, b3p=b3p, wTh=wTh, wTl=wTl, wstk=wstk, embT=embT, ones16=ones16,
                  dw1=dw1, dw2=dw2, dw3=dw3, db1=dec_b1[:, None].astype(np.float32),
                  db2p=db2p, db3p=db3p)

    in_maps = []
    for i in range(NCORES):
        sl = slice(i * NLOC, (i + 1) * NLOC)
        condT = np.ascontiguousarray(conditions[sl].T)
        chh, cll = _split_f32r(condT)
        actT = np.ascontiguousarray(actions[sl].T)
        ah, al = _split_f32r(actT)
        xmix = np.zeros((XMIX_R, NLOC), np.float32)
        xmix[0:12] = ah
        xmix[12] = 1.0
        xmix[13:25] = ah
        xmix[25] = 1.0
        xmix[32:44] = al
        m = dict(shared)
        m["condh"] = chh
        m["condl"] = cll
        m["xmix"] = xmix
        m["actl"] = np.ascontiguousarray(al)
        in_maps.append(m)

    nc = _build()
    res = run_bass_kernel_spmd(nc, in_maps, core_ids=list(range(NCORES)), trace=TRACE)
    results = res.results
    kernel._last_exec_time_ns = res.exec_time_ns
    kernel._last_results = results

    # ---- host unshard / finish ----
    reconstructed = np.concatenate([r["reconT"].T for r in results], 0)
    quantized_st = np.concatenate([r["qT"].T for r in results], 0)
    idx = np.concatenate([r["idx"] for r in results], 0).astype(np.int32)

    qsum = np.sum([r["lsums"][0:16] for r in results], axis=(0, 1))
    rsum = np.sum([r["lsums"][16:28] for r in results], axis=(0, 1))
    q_latent = np.float32(qsum / (N * E))
    e_latent = np.float32(COMMIT * (qsum / (N * E)))
    recon_loss = np.float32(rsum / (N * ACT_D))

    counts = np.bincount(idx, minlength=K).astype(np.float64)
    avg = counts / N
    perplexity = np.float32(np.exp(-np.sum(avg * np.log(avg + 1e-10))))

    A_hi = np.sum([r["cstats"][0] for r in results], 0).astype(np.float64)
    A_lo = np.sum([r["cstats"][1] for r in results], 0).astype(np.float64)
    t_hi = results[0]["cstats"][2].astype(np.float64)
    t_lo = results[0]["cstats"][3].astype(np.float64)
    n_pos = N // K
    pos = (A_hi - (N - n_pos) * t_hi) / n_pos
    S = A_lo - N / 2.0
    lse = t_lo / TAU + np.log(np.exp((pos - t_lo) / TAU) + S)
    contra = np.float32(np.mean(lse - pos / TAU))

    return (reconstructed, quantized_st, idx, q_latent, e_latent, contra,
            np.float32(perplexity), recon_loss)


kernel._last_exec_time_ns = None
kernel._last_results = None
